# revision 1
# baseline (speedup 1.0000x reference)
"""Trainium2 Bass kernel for the bipartite GNN message-passing layer.

Split: the dense node transforms (H_src @ W_src^T, H_dst @ W_dst^T — the
dominant FLOPs) run on the 8 NeuronCores, row-sharded; index gathers, the
global edge softmax, and the alpha-weighted segment sums run on the host.
"""

import os
import sys

import numpy as np

for _p in ("/opt/trn_rl_repo",):
    if _p not in sys.path and os.path.isdir(_p):
        sys.path.insert(0, _p)

N_USERS, N_ITEMS, N_NODES, N_EDGES = 50000, 20000, 70000, 320000
D = 256
NCORES = 8
P = 128
SCALE = 1.0 / float(np.sqrt(D))

UPC = N_USERS // NCORES          # 6250 users per core
IPC = N_ITEMS // NCORES          # 2500 items per core
UT = -(-UPC // P)                # 49 row tiles of 128
IT = -(-IPC // P)                # 20 row tiles
UPAD = UT * P                    # 6272
IPAD = IT * P                    # 2560

_compiled = {}
LAST = {}


def _build():
    import concourse.bacc as bacc
    import concourse.mybir as mybir
    import concourse.tile as tile

    f32 = mybir.dt.float32
    f16 = mybir.dt.float16

    nc = bacc.Bacc(
        "TRN2", target_bir_lowering=False, debug=False, num_devices=NCORES
    )
    t_hs = nc.dram_tensor("hsT", [2 * P, UPAD], f16, kind="ExternalInput")
    t_hd = nc.dram_tensor("hdT", [2 * P, IPAD], f16, kind="ExternalInput")
    t_ws = nc.dram_tensor("wsT", [2 * P, D], f16, kind="ExternalInput")
    t_wd = nc.dram_tensor("wdT", [2 * P, D], f16, kind="ExternalInput")
    t_fs = nc.dram_tensor("fs", [UPAD, D], f32, kind="ExternalOutput")
    t_fd = nc.dram_tensor("fd", [IPAD, D], f32, kind="ExternalOutput")

    with tile.TileContext(nc) as tc:
        with (
            tc.tile_pool(name="w", bufs=1) as wp,
            tc.tile_pool(name="x", bufs=4) as xp,
            tc.tile_pool(name="o", bufs=4) as op_,
            tc.tile_pool(name="ps", bufs=4, space="PSUM") as pp,
        ):
            wt = {}
            for key, tw in (("s", t_ws), ("d", t_wd)):
                w0 = wp.tile([P, D], f16, tag=f"w0{key}")
                w1 = wp.tile([P, D], f16, tag=f"w1{key}")
                nc.sync.dma_start(out=w0[:], in_=tw[0:P, :])
                nc.sync.dma_start(out=w1[:], in_=tw[P : 2 * P, :])
                wt[key] = (w0, w1)

            for key, th, tout, nt in (("s", t_hs, t_fs, UT), ("d", t_hd, t_fd, IT)):
                w0, w1 = wt[key]
                for m in range(nt):
                    x0 = xp.tile([P, P], f16, tag="x0")
                    x1 = xp.tile([P, P], f16, tag="x1")
                    sl = slice(m * P, (m + 1) * P)
                    nc.sync.dma_start(out=x0[:], in_=th[0:P, sl])
                    nc.sync.dma_start(out=x1[:], in_=th[P : 2 * P, sl])
                    ps = pp.tile([P, D], f32, tag="ps")
                    nc.tensor.matmul(
                        out=ps[:], lhsT=x0[:], rhs=w0[:], start=True, stop=False
                    )
                    nc.tensor.matmul(
                        out=ps[:], lhsT=x1[:], rhs=w1[:], start=False, stop=True
                    )
                    ob = op_.tile([P, D], f32, tag="ob")
                    nc.scalar.copy(ob[:], ps[:])
                    nc.sync.dma_start(out=tout[sl, :], in_=ob[:])
    nc.finalize()
    return nc


def kernel(**inputs):
    from concourse import bass_utils

    feat = np.asarray(inputs["feat"], np.float32)
    W_src = np.asarray(inputs["W_src"], np.float32)
    b_src = np.asarray(inputs["b_src"], np.float32)
    W_dst = np.asarray(inputs["W_dst"], np.float32)
    b_dst = np.asarray(inputs["b_dst"], np.float32)
    user_ids = np.asarray(inputs["user_ids"], np.int64)
    item_ids = np.asarray(inputs["item_ids"], np.int64)
    edge_src = np.asarray(inputs["edge_src"], np.int64)
    edge_dst = np.asarray(inputs["edge_dst"], np.int64)

    H_src = feat[user_ids]           # [U, D]
    H_dst = feat[item_ids]           # [I, D]

    # device: row-sharded dense transforms (pre-bias, pre-relu)
    hsT = np.zeros((NCORES, 2 * P, UPAD), np.float16)
    hdT = np.zeros((NCORES, 2 * P, IPAD), np.float16)
    for c in range(NCORES):
        hsT[c, :, :UPC] = H_src[c * UPC : (c + 1) * UPC].T.astype(np.float16)
        hdT[c, :, :IPC] = H_dst[c * IPC : (c + 1) * IPC].T.astype(np.float16)
    wsT = np.ascontiguousarray(W_src.T).astype(np.float16)
    wdT = np.ascontiguousarray(W_dst.T).astype(np.float16)

    if "nc" not in _compiled:
        _compiled["nc"] = _build()
    nc = _compiled["nc"]
    in_maps = [
        {"hsT": hsT[c], "hdT": hdT[c], "wsT": wsT, "wdT": wdT}
        for c in range(NCORES)
    ]
    res = bass_utils.run_bass_kernel_spmd(
        nc, in_maps, core_ids=list(range(NCORES)),
        trace=bool(os.environ.get("KERNEL_TRACE")),
    )
    LAST["results"] = res
    outs = res.results
    FS = np.concatenate([outs[c]["fs"][:UPC] for c in range(NCORES)], 0)
    FD = np.concatenate([outs[c]["fd"][:IPC] for c in range(NCORES)], 0)
    FS = np.maximum(FS + b_src[None, :], 0.0)
    FD = np.maximum(FD + b_dst[None, :], 0.0)

    # host: global edge softmax
    alpha = np.einsum(
        "ed,ed->e", H_src[edge_src], H_dst[edge_dst], optimize=True
    ) * SCALE
    w = np.exp(alpha - alpha.max())
    w /= w.sum()

    # host: alpha-weighted segment sums
    def seg_sum(vals_rows, seg_ids, nseg):
        o = np.argsort(seg_ids, kind="stable")
        seg = seg_ids[o]
        uniq, starts = np.unique(seg, return_index=True)
        sums = np.add.reduceat(vals_rows[o], starts, axis=0)
        out = np.zeros((nseg, D), np.float32)
        out[uniq] = sums
        return out

    item_new = seg_sum(FS[edge_src] * w[:, None], edge_dst, N_ITEMS)
    user_new = seg_sum(FD[edge_dst] * w[:, None], edge_src, N_USERS)
    return np.concatenate([user_new, item_new], 0).astype(np.float32)



# revision 4
# speedup vs baseline: 5.2642x; 5.2642x over previous
"""Trainium2 Bass kernel for the bipartite GNN message-passing layer.

All compute runs on the 8 NeuronCores:
  - node features are uploaded row-sharded in bf16 and AllGathered on-device
  - dense transforms (relu(H @ W^T + b)) run row-sharded on the PE array
  - edge dot-product attention (global softmax) and the alpha-weighted
    segment sums run edge-sharded: each core owns the edges whose
    destination (resp. source) it owns, so aggregation needs no collective
  - segment sums are computed as one-hot matmuls accumulated in PSUM over
    destination tiles (host pre-sorts edges by destination tile with fixed
    per-tile capacity), so no scatter-add races
  - outputs leave the device in bf16 to halve D2H traffic

Host does only O(E) integer index preprocessing and the initial
feat[user_ids]/feat[item_ids] row gathers.
"""
import os
import sys
from dataclasses import dataclass

import numpy as np

for _p in ("/opt/trn_rl_repo",):
    if _p not in sys.path and os.path.isdir(_p):
        sys.path.insert(0, _p)

import ml_dtypes

BF16 = ml_dtypes.bfloat16
P = 128
D = 256
NC = 8
SCALE = 1.0 / 16.0


@dataclass(frozen=True)
class Cfg:
    n_nodes: int
    upc: int      # users per core
    upadc: int    # padded users per core (multiple of 128)
    ipc: int      # items per core
    ipadc: int    # padded items per core (multiple of 128)
    nt_i: int     # item output tiles per core
    cpt_i: int    # chunks (128 edges) per item tile
    nt_u: int     # user output tiles per core (>= ceil(upc/128))
    cpt_u: int
    blk: int      # edges per gather block (multiple of 128)

    @property
    def ec_i(self):
        return self.nt_i * self.cpt_i * P

    @property
    def ec_u(self):
        return self.nt_u * self.cpt_u * P

    @property
    def nblk_i(self):
        return self.ec_i // self.blk

    @property
    def nblk_u(self):
        return self.ec_u // self.blk

    @property
    def sb(self):
        return self.blk // P


FULL = Cfg(
    n_nodes=70000,
    upc=6250, upadc=6272, ipc=2500, ipadc=2560,
    nt_i=20, cpt_i=20, nt_u=50, cpt_u=8, blk=2048,
)

_compiled = {}
LAST = {}


# --------------------------------------------------------------------------
# device kernel
# --------------------------------------------------------------------------

def build(cfg: Cfg):
    import concourse.bacc as bacc
    import concourse.mybir as mybir
    import concourse.tile as tile
    import concourse.bass_isa as bass_isa

    f32 = mybir.dt.float32
    bf16 = mybir.dt.bfloat16
    i16 = mybir.dt.int16
    i32 = mybir.dt.int32
    u8 = mybir.dt.uint8
    Alu = mybir.AluOpType
    Act = mybir.ActivationFunctionType
    SB = cfg.sb

    nc = bacc.Bacc("TRN2", target_bir_lowering=False, debug=False, num_devices=NC)

    t_hs = nc.dram_tensor("hs", [cfg.upadc, D], bf16, kind="ExternalInput")
    t_hd = nc.dram_tensor("hd", [cfg.ipadc, D], bf16, kind="ExternalInput")
    t_wst = nc.dram_tensor("wst", [2 * P, D], bf16, kind="ExternalInput")
    t_wdt = nc.dram_tensor("wdt", [2 * P, D], bf16, kind="ExternalInput")
    t_bs = nc.dram_tensor("bs", [1, D], f32, kind="ExternalInput")
    t_bd = nc.dram_tensor("bd", [1, D], f32, kind="ExternalInput")
    # item-direction (edges sharded by destination-item owner, sorted by tile)
    t_ihs = nc.dram_tensor("ihs", [16, cfg.ec_i // 16], i16, kind="ExternalInput")
    t_ihd = nc.dram_tensor("ihd", [16, cfg.ec_i // 16], i16, kind="ExternalInput")
    t_ipar = nc.dram_tensor("ipar", [P, cfg.ec_i // P], bf16, kind="ExternalInput")
    t_ival = nc.dram_tensor("ival", [P, cfg.ec_i // P], bf16, kind="ExternalInput")
    t_idm = nc.dram_tensor("idm", [P, cfg.ec_i // P], u8, kind="ExternalInput")
    # user-direction
    t_uhs = nc.dram_tensor("uhs", [16, cfg.ec_u // 16], i16, kind="ExternalInput")
    t_uhd = nc.dram_tensor("uhd", [16, cfg.ec_u // 16], i16, kind="ExternalInput")
    t_upar = nc.dram_tensor("upar", [P, cfg.ec_u // P], bf16, kind="ExternalInput")
    t_uval = nc.dram_tensor("uval", [P, cfg.ec_u // P], bf16, kind="ExternalInput")
    t_udm = nc.dram_tensor("udm", [P, cfg.ec_u // P], u8, kind="ExternalInput")

    t_uo = nc.dram_tensor("uo", [cfg.nt_u * P, D], bf16, kind="ExternalOutput")
    t_io = nc.dram_tensor("io", [cfg.nt_i * P, D], bf16, kind="ExternalOutput")

    CH_I = cfg.nt_i * cfg.cpt_i
    CH_U = cfg.nt_u * cfg.cpt_u

    with tile.TileContext(nc) as tc:
        with (
            tc.tile_pool(name="const", bufs=1) as cp,
            tc.tile_pool(name="idx", bufs=1) as ip,
            tc.tile_pool(name="dram", bufs=1, space="DRAM") as dr,
            tc.tile_pool(name="ps", bufs=2, space="PSUM") as pp,
            tc.tile_pool(name="gth", bufs=1) as gp_,
            tc.tile_pool(name="wrk", bufs=1) as wp,
            tc.tile_pool(name="out", bufs=2) as op_,
        ):
            # ---------------- constants / index staging ----------------
            iota_i = cp.tile([P, P], i32, tag="iota_i")
            nc.gpsimd.iota(iota_i[:], pattern=[[1, P]], base=0, channel_multiplier=0)
            iota_f = cp.tile([P, P], f32, tag="iota_f")
            nc.vector.tensor_copy(iota_f[:], iota_i[:])

            wst0 = cp.tile([P, D], bf16, tag="wst0")
            wst1 = cp.tile([P, D], bf16, tag="wst1")
            wdt0 = cp.tile([P, D], bf16, tag="wdt0")
            wdt1 = cp.tile([P, D], bf16, tag="wdt1")
            nc.sync.dma_start(out=wst0[:], in_=t_wst[0:P, :])
            nc.sync.dma_start(out=wst1[:], in_=t_wst[P : 2 * P, :])
            nc.sync.dma_start(out=wdt0[:], in_=t_wdt[0:P, :])
            nc.sync.dma_start(out=wdt1[:], in_=t_wdt[P : 2 * P, :])

            bias = {}
            for key, tb in (("s", t_bs), ("d", t_bd)):
                b1 = cp.tile([1, D], f32, tag=f"b1{key}")
                nc.sync.dma_start(out=b1[:], in_=tb[:])
                bb = cp.tile([P, D], f32, tag=f"bb{key}")
                nc.gpsimd.partition_broadcast(bb[:], b1[:])
                bias[key] = bb

            def stage_idx(th, n, tag):
                t = ip.tile([P, n // 16], i16, tag=tag)
                for k in range(8):
                    nc.sync.dma_start(out=t[16 * k : 16 * (k + 1), :], in_=th[:, :])
                return t

            ihs = stage_idx(t_ihs, cfg.ec_i, "ihs")
            ihd = stage_idx(t_ihd, cfg.ec_i, "ihd")
            uhs = stage_idx(t_uhs, cfg.ec_u, "uhs")
            uhd = stage_idx(t_uhd, cfg.ec_u, "uhd")

            def stage_pl(th, n, dt, tag):
                t = ip.tile([P, n // P], dt, tag=tag)
                nc.sync.dma_start(out=t[:], in_=th[:, :])
                return t

            ipar = stage_pl(t_ipar, cfg.ec_i, bf16, "ipar")
            ival = stage_pl(t_ival, cfg.ec_i, bf16, "ival")
            idm8 = stage_pl(t_idm, cfg.ec_i, u8, "idm8")
            upar = stage_pl(t_upar, cfg.ec_u, bf16, "upar")
            uval = stage_pl(t_uval, cfg.ec_u, bf16, "uval")
            udm8 = stage_pl(t_udm, cfg.ec_u, u8, "udm8")

            idmf = ip.tile([P, CH_I], f32, tag="idmf")
            nc.vector.tensor_copy(idmf[:], idm8[:])
            udmf = ip.tile([P, CH_U], f32, tag="udmf")
            nc.vector.tensor_copy(udmf[:], udm8[:])

            # ---------------- AllGather H tables ----------------
            hs_b = dr.tile([cfg.upadc, D], bf16, tag="hs_b")
            hd_b = dr.tile([cfg.ipadc, D], bf16, tag="hd_b")
            nc.gpsimd.dma_start(out=hs_b[:], in_=t_hs[:])
            nc.gpsimd.dma_start(out=hd_b[:], in_=t_hd[:])
            HsF = dr.tile([NC * cfg.upadc, D], bf16, tag="HsF")
            HdF = dr.tile([NC * cfg.ipadc, D], bf16, tag="HdF")
            nc.gpsimd.collective_compute(
                "AllGather", Alu.bypass, replica_groups=[list(range(NC))],
                ins=[hs_b[:].opt()], outs=[HsF[:].opt()],
            )
            nc.gpsimd.collective_compute(
                "AllGather", Alu.bypass, replica_groups=[list(range(NC))],
                ins=[hd_b[:].opt()], outs=[HdF[:].opt()],
            )
            HsP = HsF[:].rearrange("(a b) c -> a (b c)", b=2)
            HdA = HdF[:]

            # ---------------- dense transforms ----------------
            fs_sh = dr.tile([cfg.upadc, D], bf16, tag="fs_sh")
            fd_sh = dr.tile([cfg.ipadc, D], bf16, tag="fd_sh")

            for key, t_in, npad, w0, w1, f_out in (
                ("s", t_hs, cfg.upadc, wst0, wst1, fs_sh),
                ("d", t_hd, cfg.ipadc, wdt0, wdt1, fd_sh),
            ):
                htA = cp.tile([P, npad], bf16, tag=f"htA{key}")
                htB = cp.tile([P, npad], bf16, tag=f"htB{key}")
                nc.sync.dma_start_transpose(htA[:], t_in[:, 0:P])
                nc.sync.dma_start_transpose(htB[:], t_in[:, P : 2 * P])
                for t in range(npad // P):
                    ps = pp.tile([P, D], f32, tag="mmps")
                    nc.tensor.matmul(
                        out=ps[:], lhsT=htA[:, t * P : (t + 1) * P], rhs=w0[:],
                        start=True, stop=False,
                    )
                    nc.tensor.matmul(
                        out=ps[:], lhsT=htB[:, t * P : (t + 1) * P], rhs=w1[:],
                        start=False, stop=True,
                    )
                    tmp = op_.tile([P, D], f32, tag="mmtmp")
                    nc.vector.tensor_tensor(tmp[:], ps[:], bias[key][:], Alu.add)
                    ft = op_.tile([P, D], bf16, tag="mmft")
                    nc.vector.tensor_scalar_max(ft[:], tmp[:], 0.0)
                    nc.sync.dma_start(out=f_out[t * P : (t + 1) * P, :], in_=ft[:])

            FsF = dr.tile([NC * cfg.upadc, D], bf16, tag="FsF")
            FdF = dr.tile([NC * cfg.ipadc, D], bf16, tag="FdF")
            nc.gpsimd.collective_compute(
                "AllGather", Alu.bypass, replica_groups=[list(range(NC))],
                ins=[fs_sh[:].opt()], outs=[FsF[:].opt()],
            )
            nc.gpsimd.collective_compute(
                "AllGather", Alu.bypass, replica_groups=[list(range(NC))],
                ins=[fd_sh[:].opt()], outs=[FdF[:].opt()],
            )
            FsP = FsF[:].rearrange("(a b) c -> a (b c)", b=2)
            FdA = FdF[:]

            # ---------------- item-direction alphas ----------------
            alpha_i = ip.tile([P, CH_I], f32, tag="alpha_i")

            GCALL = 1024  # max idxs per dma_gather (16-DMA ring: 128 descs)

            def emit_gather(dst, table, idxt, base_col, blk, elem):
                n = 0
                while n < blk:
                    step = min(GCALL, blk - n)
                    nc.gpsimd.dma_gather(
                        dst[:, n // P : (n + step) // P, :],
                        table,
                        idxt[:, base_col + n // 16 : base_col + (n + step) // 16],
                        step, step, elem,
                    )
                    n += step

            def pair_select(gpt, par_sl, tag):
                """gpt [P,SB,2D] pair-gather; returns selected [P,SB,D] bf16."""
                sel = wp.tile([P, SB, D], bf16, tag=tag)
                nc.vector.tensor_tensor(
                    sel[:], gpt[:, :, D : 2 * D], gpt[:, :, 0:D], Alu.subtract
                )
                nc.vector.tensor_tensor(
                    sel[:], sel[:],
                    par_sl[:, :, None].to_broadcast((P, SB, D)), Alu.mult,
                )
                nc.vector.tensor_tensor(sel[:], sel[:], gpt[:, :, 0:D], Alu.add)
                return sel

            for b in range(cfg.nblk_i):
                i16sl = slice(b * (cfg.blk // 16), (b + 1) * (cfg.blk // 16))
                chsl = slice(b * SB, (b + 1) * SB)
                gpt = gp_.tile([P, SB, 2 * D], bf16, tag="gp")
                emit_gather(gpt, HsP, ihs, b * (cfg.blk // 16), cfg.blk, 2 * D)
                sel = pair_select(gpt, ipar[:, chsl], "sel")
                gb = gp_.tile([P, SB, D], bf16, tag="gb")
                emit_gather(gb, HdA, ihd, b * (cfg.blk // 16), cfg.blk, D)
                nc.vector.tensor_tensor(gb[:], sel[:], gb[:], Alu.mult)
                nc.vector.tensor_reduce(
                    alpha_i[:, chsl], gb[:], mybir.AxisListType.X, Alu.add
                )

            # ---------------- global softmax stats ----------------
            lred = cp.tile([P, 1], f32, tag="lred")
            pred = cp.tile([P, 1], f32, tag="pred")
            nc.vector.tensor_reduce(
                lred[:], alpha_i[:], mybir.AxisListType.X, Alu.max
            )
            nc.gpsimd.partition_all_reduce(
                pred[:], lred[:], channels=P, reduce_op=bass_isa.ReduceOp.max
            )
            cc_in = dr.tile([1, 1], f32, tag="cc_in")
            cc_out = dr.tile([1, 1], f32, tag="cc_out")
            nc.gpsimd.dma_start(out=cc_in[:], in_=pred[0:1, 0:1])
            nc.gpsimd.collective_compute(
                "AllReduce", Alu.max, replica_groups=[list(range(NC))],
                ins=[cc_in[:].opt()], outs=[cc_out[:].opt()],
            )
            gmax1 = cp.tile([1, 1], f32, tag="gmax1")
            nc.sync.dma_start(out=gmax1[:], in_=cc_out[:])
            negb = cp.tile([P, 1], f32, tag="negb")
            nc.gpsimd.partition_broadcast(negb[:], gmax1[:])
            nc.vector.tensor_scalar_mul(negb[:], negb[:], -SCALE)

            w_i = ip.tile([P, CH_I], f32, tag="w_i")
            nc.scalar.activation(
                w_i[:], alpha_i[:], Act.Exp, bias=negb[:], scale=SCALE
            )
            nc.vector.tensor_tensor(w_i[:], w_i[:], ival[:], Alu.mult)
            lsum = cp.tile([P, 1], f32, tag="lsum")
            psum_ = cp.tile([P, 1], f32, tag="psum_")
            nc.vector.tensor_reduce(
                lsum[:], w_i[:], mybir.AxisListType.X, Alu.add
            )
            nc.gpsimd.partition_all_reduce(
                psum_[:], lsum[:], channels=P, reduce_op=bass_isa.ReduceOp.add
            )
            z_in = dr.tile([1, 1], f32, tag="z_in")
            z_out = dr.tile([1, 1], f32, tag="z_out")
            nc.gpsimd.dma_start(out=z_in[:], in_=psum_[0:1, 0:1])
            nc.gpsimd.collective_compute(
                "AllReduce", Alu.add, replica_groups=[list(range(NC))],
                ins=[z_in[:].opt()], outs=[z_out[:].opt()],
            )
            zt = cp.tile([1, 1], f32, tag="zt")
            nc.sync.dma_start(out=zt[:], in_=z_out[:])
            invz1 = cp.tile([1, 1], f32, tag="invz1")
            nc.vector.reciprocal(invz1[:], zt[:])
            invz = cp.tile([P, 1], f32, tag="invz")
            nc.gpsimd.partition_broadcast(invz[:], invz1[:])
            nc.vector.tensor_tensor(
                w_i[:], w_i[:], invz[:].to_broadcast((P, CH_I)), Alu.mult
            )

            # ---------------- item-direction aggregation ----------------
            def agg_blocks(nblk, cpt, w_src, dm_f, sel_fn, t_out, tag_pfx):
                ps = None
                for b in range(nblk):
                    sel, wsl = sel_fn(b)
                    eqw = wp.tile([P, SB, P], bf16, tag=f"{tag_pfx}eqw")
                    nc.vector.tensor_tensor(
                        eqw[:],
                        dm_f[:, b * SB : (b + 1) * SB, None].to_broadcast((P, SB, P)),
                        iota_f[:, None, :].to_broadcast((P, SB, P)),
                        Alu.is_equal,
                    )
                    nc.vector.tensor_tensor(
                        eqw[:], eqw[:], wsl, Alu.mult
                    )
                    for sl in range(SB):
                        g = b * SB + sl
                        t, c = divmod(g, cpt)
                        if c == 0:
                            ps = pp.tile([P, D], f32, tag=f"{tag_pfx}ps")
                        nc.tensor.matmul(
                            out=ps[:], lhsT=eqw[:, sl, :], rhs=sel[:, sl, :],
                            start=(c == 0), stop=(c == cpt - 1),
                        )
                        if c == cpt - 1:
                            ob = op_.tile([P, D], bf16, tag=f"{tag_pfx}ob")
                            nc.scalar.copy(ob[:], ps[:])
                            nc.sync.dma_start(
                                out=t_out[t * P : (t + 1) * P, :], in_=ob[:]
                            )

            def item_sel(b):
                i16sl = slice(b * (cfg.blk // 16), (b + 1) * (cfg.blk // 16))
                chsl = slice(b * SB, (b + 1) * SB)
                gpt = gp_.tile([P, SB, 2 * D], bf16, tag="gp")
                emit_gather(gpt, FsP, ihs, b * (cfg.blk // 16), cfg.blk, 2 * D)
                sel = pair_select(gpt, ipar[:, chsl], "sel")
                wsl = w_i[:, chsl, None].to_broadcast((P, SB, P))
                return sel, wsl

            agg_blocks(cfg.nblk_i, cfg.cpt_i, w_i, idmf, item_sel, t_io, "i")

            # ---------------- user-direction (alpha fused) ----------------
            w_ub = {}

            def user_sel(b):
                i16sl = slice(b * (cfg.blk // 16), (b + 1) * (cfg.blk // 16))
                chsl = slice(b * SB, (b + 1) * SB)
                gpt = gp_.tile([P, SB, 2 * D], bf16, tag="gp")
                emit_gather(gpt, HsP, uhs, b * (cfg.blk // 16), cfg.blk, 2 * D)
                sel = pair_select(gpt, upar[:, chsl], "sel")
                gb = gp_.tile([P, SB, D], bf16, tag="gb")
                emit_gather(gb, HdA, uhd, b * (cfg.blk // 16), cfg.blk, D)
                nc.vector.tensor_tensor(gb[:], sel[:], gb[:], Alu.mult)
                aub = wp.tile([P, SB], f32, tag="aub")
                nc.vector.tensor_reduce(
                    aub[:], gb[:], mybir.AxisListType.X, Alu.add
                )
                wub = wp.tile([P, SB], f32, tag="wub")
                nc.scalar.activation(
                    wub[:], aub[:], Act.Exp, bias=negb[:], scale=SCALE
                )
                nc.vector.tensor_tensor(
                    wub[:], wub[:], uval[:, chsl], Alu.mult
                )
                nc.vector.tensor_tensor(
                    wub[:], wub[:], invz[:].to_broadcast((P, SB)), Alu.mult
                )
                gfd = gp_.tile([P, SB, D], bf16, tag="gfd")
                emit_gather(gfd, FdA, uhd, b * (cfg.blk // 16), cfg.blk, D)
                wsl = wub[:, :, None].to_broadcast((P, SB, P))
                return gfd, wsl

            agg_blocks(cfg.nblk_u, cfg.cpt_u, None, udmf, user_sel, t_uo, "u")

    nc.finalize()
    return nc


# --------------------------------------------------------------------------
# host preprocessing
# --------------------------------------------------------------------------

def wrap16(a):
    """per-edge int array [NCORES, n] -> dma_gather 16-wrap layout [NC,16,n/16]."""
    ncore, n = a.shape
    return np.ascontiguousarray(
        a.reshape(ncore, n // 16, 16).transpose(0, 2, 1)
    ).astype(np.int16)


def glayout(a):
    """per-edge array [NCORES, n] -> gather-output layout [NC, 128, n/128]."""
    ncore, n = a.shape
    return np.ascontiguousarray(a.reshape(ncore, n // P, P).transpose(0, 2, 1))


def prep_direction(cfg: Cfg, e_own_ids, e_oth_ids, own_per_core, nt, cpt,
                   own_pad, oth_per_core, oth_pad, own_is_user):
    """Sort/pad one direction's edges by (owner core, dest tile).

    e_own_ids: destination-side node ids (ownership + one-hot row)
    e_oth_ids: gathered-side node ids
    Returns dict of per-core arrays + ok flag.
    """
    E = e_own_ids.shape[0]
    ec = nt * cpt * P
    own = e_own_ids // own_per_core
    loc = e_own_ids - own * own_per_core
    til = loc >> 7
    grp = own * nt + til
    order = np.argsort(grp, kind="stable")
    grp_s = grp[order]
    counts = np.bincount(grp, minlength=NC * nt)
    if counts.max() > cpt * P:
        return None
    starts = np.concatenate([[0], np.cumsum(counts)[:-1]])
    rank = np.arange(E, dtype=np.int64) - starts[grp_s]
    slot = (grp_s % nt) * (cpt * P) + rank
    core = grp_s // nt
    loc_s = loc[order]
    oth_s = e_oth_ids[order]
    own_row = (e_own_ids[order] // own_per_core) * own_pad + loc_s
    oth_row = (oth_s // oth_per_core) * oth_pad + (oth_s % oth_per_core)

    if own_is_user:
        hs_row, hd_row = own_row, oth_row
    else:
        hs_row, hd_row = oth_row, own_row

    hs_idx = np.zeros((NC, ec), np.int64)
    hd_idx = np.zeros((NC, ec), np.int64)
    par = np.zeros((NC, ec), np.float32)
    val = np.zeros((NC, ec), np.float32)
    dmod = np.zeros((NC, ec), np.int64)
    hs_idx[core, slot] = hs_row >> 1
    par[core, slot] = hs_row & 1
    hd_idx[core, slot] = hd_row
    dmod[core, slot] = loc_s & 127
    val[core, slot] = 1.0
    return {
        "hs": wrap16(hs_idx),
        "hd": wrap16(hd_idx),
        "par": glayout(par).astype(BF16),
        "val": glayout(val).astype(BF16),
        "dm": glayout(dmod).astype(np.uint8),
    }


def host_prep(cfg: Cfg, H_src, H_dst, W_src, b_src, W_dst, b_dst,
              edge_src, edge_dst):
    hs_bf = H_src.astype(BF16)
    hd_bf = H_dst.astype(BF16)
    hs_sh = np.zeros((NC, cfg.upadc, D), BF16)
    hs_sh[:, : cfg.upc] = hs_bf.reshape(NC, cfg.upc, D)
    hd_sh = np.zeros((NC, cfg.ipadc, D), BF16)
    hd_sh[:, : cfg.ipc] = hd_bf.reshape(NC, cfg.ipc, D)

    idir = prep_direction(
        cfg, edge_dst, edge_src, cfg.ipc, cfg.nt_i, cfg.cpt_i,
        cfg.ipadc, cfg.upc, cfg.upadc, own_is_user=False,
    )
    udir = prep_direction(
        cfg, edge_src, edge_dst, cfg.upc, cfg.nt_u, cfg.cpt_u,
        cfg.upadc, cfg.ipc, cfg.ipadc, own_is_user=True,
    )
    if idir is None or udir is None:
        return None

    wst = np.ascontiguousarray(W_src.T).astype(BF16)
    wdt = np.ascontiguousarray(W_dst.T).astype(BF16)

    def rep(a):
        return np.broadcast_to(a, (NC, *a.shape))

    ins = {
        "hs": hs_sh, "hd": hd_sh,
        "wst": rep(wst), "wdt": rep(wdt),
        "bs": rep(b_src.reshape(1, D).astype(np.float32)),
        "bd": rep(b_dst.reshape(1, D).astype(np.float32)),
        "ihs": idir["hs"], "ihd": idir["hd"], "ipar": idir["par"],
        "ival": idir["val"], "idm": idir["dm"],
        "uhs": udir["hs"], "uhd": udir["hd"], "upar": udir["par"],
        "uval": udir["val"], "udm": udir["dm"],
    }
    return {k: np.ascontiguousarray(v.reshape(-1, *v.shape[2:])) for k, v in ins.items()}


# --------------------------------------------------------------------------
# cached SPMD runner (jit built once, zeros created on-device)
# --------------------------------------------------------------------------

class SpmdRunner:
    def __init__(self, nc, n_cores):
        import jax
        import jax.numpy as jnp
        from jax.sharding import Mesh, NamedSharding, PartitionSpec
        from jax.experimental.shard_map import shard_map
        from concourse import mybir
        from concourse.bass2jax import (
            _bass_exec_p, partition_id_tensor, install_neuronx_cc_hook,
        )

        install_neuronx_cc_hook()
        partition_name = (
            nc.partition_id_tensor.name if nc.partition_id_tensor else None
        )
        in_names, out_names, out_avals, zero_shapes = [], [], [], []
        for alloc in nc.m.functions[0].allocations:
            if not isinstance(alloc, mybir.MemoryLocationSet):
                continue
            name = alloc.memorylocations[0].name
            if alloc.kind == "ExternalInput":
                if name != partition_name:
                    in_names.append(name)
            elif alloc.kind == "ExternalOutput":
                out_names.append(name)
                shape = tuple(alloc.tensor_shape)
                dtype = mybir.dt.np(alloc.dtype)
                out_avals.append(jax.core.ShapedArray(shape, dtype))
                zero_shapes.append((shape, dtype))
        self.in_names = in_names
        self.out_names = out_names
        n_params = len(in_names)
        n_outs = len(out_avals)
        all_in = list(in_names) + list(out_names)
        if partition_name is not None:
            all_in.append(partition_name)
        donate = tuple(range(n_params, n_params + n_outs))

        def _body(*args):
            operands = list(args)
            if partition_name is not None:
                operands.append(partition_id_tensor())
            outs = _bass_exec_p.bind(
                *operands,
                out_avals=tuple(out_avals),
                in_names=tuple(all_in),
                out_names=tuple(out_names),
                lowering_input_output_aliases=(),
                sim_require_finite=False,
                sim_require_nnan=False,
                nc=nc,
            )
            return tuple(outs)

        devices = jax.devices()[:n_cores]
        mesh = Mesh(np.asarray(devices), ("core",))
        in_specs = (PartitionSpec("core"),) * (n_params + n_outs)
        out_specs = (PartitionSpec("core"),) * n_outs
        self.sharded = jax.jit(
            shard_map(
                _body, mesh=mesh, in_specs=in_specs, out_specs=out_specs,
                check_rep=False,
            ),
            donate_argnums=donate,
            keep_unused=True,
        )
        shd = NamedSharding(mesh, PartitionSpec("core"))
        self.zeros_fn = jax.jit(
            lambda: tuple(
                jnp.zeros((n_cores * s[0], *s[1:]), d) for s, d in zero_shapes
            ),
            out_shardings=(shd,) * n_outs,
        )

    def __call__(self, stacked_inputs):
        args = [stacked_inputs[n] for n in self.in_names]
        zeros = self.zeros_fn()
        outs = self.sharded(*args, *zeros)
        return dict(zip(self.out_names, outs))


# --------------------------------------------------------------------------
# fallback: pure-host computation (only if capacity asserts fail)
# --------------------------------------------------------------------------

def _host_fallback(feat, W_src, b_src, W_dst, b_dst, user_ids, item_ids,
                   edge_src, edge_dst):
    H_src = feat[user_ids]
    H_dst = feat[item_ids]
    alpha = np.einsum(
        "ed,ed->e", H_src[edge_src], H_dst[edge_dst], optimize=True
    ) * SCALE
    w = np.exp(alpha - alpha.max())
    w /= w.sum()
    FS = np.maximum(H_src @ W_src.T + b_src[None, :], 0.0)
    FD = np.maximum(H_dst @ W_dst.T + b_dst[None, :], 0.0)

    def seg_sum(vals, seg, nseg):
        out = np.zeros((nseg, D), np.float32)
        np.add.at(out, seg, vals)
        return out

    item_new = seg_sum(FS[edge_src] * w[:, None], edge_dst, len(item_ids))
    user_new = seg_sum(FD[edge_dst] * w[:, None], edge_src, len(user_ids))
    return np.concatenate([user_new, item_new], 0).astype(np.float32)


# --------------------------------------------------------------------------
# entry point
# --------------------------------------------------------------------------

def kernel(**inputs):
    cfg = FULL
    feat = np.asarray(inputs["feat"], np.float32)
    W_src = np.asarray(inputs["W_src"], np.float32)
    b_src = np.asarray(inputs["b_src"], np.float32)
    W_dst = np.asarray(inputs["W_dst"], np.float32)
    b_dst = np.asarray(inputs["b_dst"], np.float32)
    user_ids = np.asarray(inputs["user_ids"]).astype(np.int64)
    item_ids = np.asarray(inputs["item_ids"]).astype(np.int64)
    edge_src = np.asarray(inputs["edge_src"]).astype(np.int64)
    edge_dst = np.asarray(inputs["edge_dst"]).astype(np.int64)

    H_src = feat[user_ids]
    H_dst = feat[item_ids]

    ins = host_prep(
        cfg, H_src, H_dst, W_src, b_src, W_dst, b_dst, edge_src, edge_dst
    )
    if ins is None:
        return _host_fallback(
            feat, W_src, b_src, W_dst, b_dst, user_ids, item_ids,
            edge_src, edge_dst,
        )

    if "runner" not in _compiled:
        nc = build(cfg)
        _compiled["runner"] = SpmdRunner(nc, NC)
    runner = _compiled["runner"]
    outs = runner(ins)
    LAST["results"] = None

    uo = np.asarray(outs["uo"]).reshape(NC, cfg.nt_u * P, D)[:, : cfg.upc]
    io = np.asarray(outs["io"]).reshape(NC, cfg.nt_i * P, D)[:, : cfg.ipc]
    user_new = uo.reshape(NC * cfg.upc, D).astype(np.float32)
    item_new = io.reshape(NC * cfg.ipc, D).astype(np.float32)
    return np.concatenate([user_new, item_new], 0)


# revision 8
# speedup vs baseline: 8.0807x; 1.5350x over previous
"""Trainium2 Bass kernel for the bipartite GNN message-passing layer.

All compute runs on the 8 NeuronCores:
  - node features are uploaded row-sharded in bf16 and AllGathered on-device
  - dense transforms (relu(H @ W^T + b)) run row-sharded on the PE array
  - edge dot-product attention (global softmax) and the alpha-weighted
    segment sums run edge-sharded: each core owns the edges whose
    destination (resp. source) it owns, so aggregation needs no collective
  - segment sums are computed as one-hot matmuls accumulated in PSUM over
    destination tiles (host pre-sorts edges by destination tile with fixed
    per-tile capacity), so no scatter-add races
  - outputs leave the device in bf16 to halve D2H traffic

Host does only O(E) integer index preprocessing and the initial
feat[user_ids]/feat[item_ids] row gathers.
"""
import os
import sys
from dataclasses import dataclass

import numpy as np

for _p in ("/opt/trn_rl_repo",):
    if _p not in sys.path and os.path.isdir(_p):
        sys.path.insert(0, _p)

import ml_dtypes

BF16 = ml_dtypes.bfloat16
P = 128
D = 256
NC = 8
SCALE = 1.0 / 16.0


@dataclass(frozen=True)
class Cfg:
    n_nodes: int
    upc: int      # users per core
    upadc: int    # padded users per core (multiple of 128)
    ipc: int      # items per core
    ipadc: int    # padded items per core (multiple of 128)
    nt_i: int     # item output tiles per core
    cpt_i: int    # chunks (128 edges) per item tile
    nt_u: int     # user output tiles per core (>= ceil(upc/128))
    cpt_u: int
    blk: int      # edges per gather block (multiple of 128)
    gtu: int      # deduped user-node gather-table rows per core (mult of 128)
    gti: int      # deduped item-node gather-table rows per core (mult of 128)

    @property
    def ec_i(self):
        return self.nt_i * self.cpt_i * P

    @property
    def ec_u(self):
        return self.nt_u * self.cpt_u * P

    @property
    def nblk_i(self):
        return self.ec_i // self.blk

    @property
    def nblk_u(self):
        return self.ec_u // self.blk

    @property
    def sb(self):
        return self.blk // P


FULL = Cfg(
    n_nodes=70000,
    upc=6250, upadc=6272, ipc=2500, ipadc=2560,
    nt_i=20, cpt_i=20, nt_u=50, cpt_u=8, blk=2048,
    gtu=4608, gti=2304,
)

_compiled = {}
LAST = {}


# --------------------------------------------------------------------------
# device kernel
# --------------------------------------------------------------------------

def build(cfg: Cfg):
    import concourse.bacc as bacc
    import concourse.mybir as mybir
    import concourse.tile as tile
    import concourse.bass_isa as bass_isa

    f32 = mybir.dt.float32
    bf16 = mybir.dt.bfloat16
    i16 = mybir.dt.int16
    i32 = mybir.dt.int32
    u8 = mybir.dt.uint8
    Alu = mybir.AluOpType
    Act = mybir.ActivationFunctionType
    SB = cfg.sb

    nc = bacc.Bacc("TRN2", target_bir_lowering=False, debug=False, num_devices=NC)

    t_hs = nc.dram_tensor("hs", [cfg.gtu, D], bf16, kind="ExternalInput")
    t_hd = nc.dram_tensor("hd", [cfg.gti, D], bf16, kind="ExternalInput")
    t_wst = nc.dram_tensor("wst", [2 * P, D], bf16, kind="ExternalInput")
    t_wdt = nc.dram_tensor("wdt", [2 * P, D], bf16, kind="ExternalInput")
    t_bs = nc.dram_tensor("bs", [1, D], f32, kind="ExternalInput")
    t_bd = nc.dram_tensor("bd", [1, D], f32, kind="ExternalInput")
    # item-direction (edges sharded by destination-item owner, sorted by tile)
    t_ihs = nc.dram_tensor("ihs", [16, cfg.ec_i // 16], i16, kind="ExternalInput")
    t_ihd = nc.dram_tensor("ihd", [16, cfg.ec_i // 16], i16, kind="ExternalInput")
    t_ipar = nc.dram_tensor("ipar", [P, cfg.ec_i // P], bf16, kind="ExternalInput")
    t_ival = nc.dram_tensor("ival", [P, cfg.ec_i // P], bf16, kind="ExternalInput")
    t_idm = nc.dram_tensor("idm", [P, cfg.ec_i // P], u8, kind="ExternalInput")
    # user-direction
    t_uhs = nc.dram_tensor("uhs", [16, cfg.ec_u // 16], i16, kind="ExternalInput")
    t_uhd = nc.dram_tensor("uhd", [16, cfg.ec_u // 16], i16, kind="ExternalInput")
    t_upar = nc.dram_tensor("upar", [P, cfg.ec_u // P], bf16, kind="ExternalInput")
    t_uval = nc.dram_tensor("uval", [P, cfg.ec_u // P], bf16, kind="ExternalInput")
    t_udm = nc.dram_tensor("udm", [P, cfg.ec_u // P], u8, kind="ExternalInput")

    i8 = mybir.dt.int8
    t_uo = nc.dram_tensor("uo", [cfg.nt_u * P, D], i8, kind="ExternalOutput")
    t_io = nc.dram_tensor("io", [cfg.nt_i * P, D], i8, kind="ExternalOutput")
    t_us = nc.dram_tensor("us", [cfg.nt_u * P, 1], f32, kind="ExternalOutput")
    t_is = nc.dram_tensor("is_", [cfg.nt_i * P, 1], f32, kind="ExternalOutput")

    CH_I = cfg.nt_i * cfg.cpt_i
    CH_U = cfg.nt_u * cfg.cpt_u

    with tile.TileContext(nc) as tc:
        with (
            tc.tile_pool(name="const", bufs=1) as cp,
            tc.tile_pool(name="idx", bufs=1) as ip,
            tc.tile_pool(name="dram", bufs=1, space="DRAM") as dr,
            tc.tile_pool(name="ps", bufs=2, space="PSUM") as pp,
            tc.tile_pool(name="gth", bufs=1) as gp_,
            tc.tile_pool(name="wrk", bufs=1) as wp,
            tc.tile_pool(name="out", bufs=2) as op_,
        ):
            # ---------------- constants / index staging ----------------
            iota_i = cp.tile([P, P], i32, tag="iota_i")
            nc.gpsimd.iota(iota_i[:], pattern=[[1, P]], base=0, channel_multiplier=0)
            iota_f = cp.tile([P, P], f32, tag="iota_f")
            nc.vector.tensor_copy(iota_f[:], iota_i[:])

            wst0 = cp.tile([P, D], bf16, tag="wst0")
            wst1 = cp.tile([P, D], bf16, tag="wst1")
            wdt0 = cp.tile([P, D], bf16, tag="wdt0")
            wdt1 = cp.tile([P, D], bf16, tag="wdt1")
            nc.sync.dma_start(out=wst0[:], in_=t_wst[0:P, :])
            nc.sync.dma_start(out=wst1[:], in_=t_wst[P : 2 * P, :])
            nc.sync.dma_start(out=wdt0[:], in_=t_wdt[0:P, :])
            nc.sync.dma_start(out=wdt1[:], in_=t_wdt[P : 2 * P, :])

            bias = {}
            for key, tb in (("s", t_bs), ("d", t_bd)):
                b1 = cp.tile([1, D], f32, tag=f"b1{key}")
                nc.sync.dma_start(out=b1[:], in_=tb[:])
                bb = cp.tile([P, D], f32, tag=f"bb{key}")
                nc.gpsimd.partition_broadcast(bb[:], b1[:])
                bias[key] = bb

            def stage_idx(th, n, tag):
                t = ip.tile([P, n // 16], i16, tag=tag)
                for k in range(8):
                    nc.sync.dma_start(out=t[16 * k : 16 * (k + 1), :], in_=th[:, :])
                return t

            ihs = stage_idx(t_ihs, cfg.ec_i, "ihs")
            ihd = stage_idx(t_ihd, cfg.ec_i, "ihd")
            uhs = stage_idx(t_uhs, cfg.ec_u, "uhs")
            uhd = stage_idx(t_uhd, cfg.ec_u, "uhd")

            def stage_pl(th, n, dt, tag):
                t = ip.tile([P, n // P], dt, tag=tag)
                nc.sync.dma_start(out=t[:], in_=th[:, :])
                return t

            ipar = stage_pl(t_ipar, cfg.ec_i, bf16, "ipar")
            ival = stage_pl(t_ival, cfg.ec_i, bf16, "ival")
            idm8 = stage_pl(t_idm, cfg.ec_i, u8, "idm8")
            upar = stage_pl(t_upar, cfg.ec_u, bf16, "upar")
            uval = stage_pl(t_uval, cfg.ec_u, bf16, "uval")
            udm8 = stage_pl(t_udm, cfg.ec_u, u8, "udm8")

            idmf = ip.tile([P, CH_I], f32, tag="idmf")
            nc.vector.tensor_copy(idmf[:], idm8[:])
            udmf = ip.tile([P, CH_U], f32, tag="udmf")
            nc.vector.tensor_copy(udmf[:], udm8[:])

            # ---------------- AllGather H tables ----------------
            hs_b = dr.tile([cfg.gtu, D], bf16, tag="hs_b")
            hd_b = dr.tile([cfg.gti, D], bf16, tag="hd_b")
            nc.gpsimd.dma_start(out=hs_b[:], in_=t_hs[:])
            nc.gpsimd.dma_start(out=hd_b[:], in_=t_hd[:])
            HsF = dr.tile([NC * cfg.gtu, D], bf16, tag="HsF")
            HdF = dr.tile([NC * cfg.gti, D], bf16, tag="HdF")
            nc.gpsimd.collective_compute(
                "AllGather", Alu.bypass, replica_groups=[list(range(NC))],
                ins=[hs_b[:].opt()], outs=[HsF[:].opt()],
            )
            nc.gpsimd.collective_compute(
                "AllGather", Alu.bypass, replica_groups=[list(range(NC))],
                ins=[hd_b[:].opt()], outs=[HdF[:].opt()],
            )
            HsP = HsF[:].rearrange("(a b) c -> a (b c)", b=2)
            HdA = HdF[:]

            # ---------------- dense transforms ----------------
            fs_sh = dr.tile([cfg.gtu, D], bf16, tag="fs_sh")
            fd_sh = dr.tile([cfg.gti, D], bf16, tag="fd_sh")

            for key, t_in, npad, w0, w1, f_out in (
                ("s", t_hs, cfg.gtu, wst0, wst1, fs_sh),
                ("d", t_hd, cfg.gti, wdt0, wdt1, fd_sh),
            ):
                htA = cp.tile([P, npad], bf16, tag=f"htA{key}")
                htB = cp.tile([P, npad], bf16, tag=f"htB{key}")
                nc.sync.dma_start_transpose(htA[:], t_in[:, 0:P])
                nc.sync.dma_start_transpose(htB[:], t_in[:, P : 2 * P])
                for t in range(npad // P):
                    ps = pp.tile([P, D], f32, tag="mmps")
                    nc.tensor.matmul(
                        out=ps[:], lhsT=htA[:, t * P : (t + 1) * P], rhs=w0[:],
                        start=True, stop=False,
                    )
                    nc.tensor.matmul(
                        out=ps[:], lhsT=htB[:, t * P : (t + 1) * P], rhs=w1[:],
                        start=False, stop=True,
                    )
                    tmp = op_.tile([P, D], f32, tag="mmtmp")
                    nc.vector.tensor_tensor(tmp[:], ps[:], bias[key][:], Alu.add)
                    ft = op_.tile([P, D], bf16, tag="mmft")
                    nc.vector.tensor_scalar_max(ft[:], tmp[:], 0.0)
                    nc.sync.dma_start(out=f_out[t * P : (t + 1) * P, :], in_=ft[:])

            FsF = dr.tile([NC * cfg.gtu, D], bf16, tag="FsF")
            FdF = dr.tile([NC * cfg.gti, D], bf16, tag="FdF")
            nc.gpsimd.collective_compute(
                "AllGather", Alu.bypass, replica_groups=[list(range(NC))],
                ins=[fs_sh[:].opt()], outs=[FsF[:].opt()],
            )
            nc.gpsimd.collective_compute(
                "AllGather", Alu.bypass, replica_groups=[list(range(NC))],
                ins=[fd_sh[:].opt()], outs=[FdF[:].opt()],
            )
            FsP = FsF[:].rearrange("(a b) c -> a (b c)", b=2)
            FdA = FdF[:]

            # ---------------- item-direction alphas ----------------
            alpha_i = ip.tile([P, CH_I], f32, tag="alpha_i")

            GCALL = 1024  # max idxs per dma_gather (16-DMA ring: 128 descs)

            def emit_gather(dst, table, idxt, base_col, blk, elem):
                n = 0
                while n < blk:
                    step = min(GCALL, blk - n)
                    nc.gpsimd.dma_gather(
                        dst[:, n // P : (n + step) // P, :],
                        table,
                        idxt[:, base_col + n // 16 : base_col + (n + step) // 16],
                        step, step, elem,
                    )
                    n += step

            def pair_select(gpt, par_sl, tag):
                """gpt [P,SB,2D] pair-gather; returns selected [P,SB,D] bf16."""
                sel = wp.tile([P, SB, D], bf16, tag=tag)
                nc.vector.tensor_tensor(
                    sel[:], gpt[:, :, D : 2 * D], gpt[:, :, 0:D], Alu.subtract
                )
                nc.vector.tensor_tensor(
                    sel[:], sel[:],
                    par_sl[:, :, None].to_broadcast((P, SB, D)), Alu.mult,
                )
                nc.vector.tensor_tensor(sel[:], sel[:], gpt[:, :, 0:D], Alu.add)
                return sel

            for b in range(cfg.nblk_i):
                i16sl = slice(b * (cfg.blk // 16), (b + 1) * (cfg.blk // 16))
                chsl = slice(b * SB, (b + 1) * SB)
                gpt = gp_.tile([P, SB, 2 * D], bf16, tag="gp")
                emit_gather(gpt, HsP, ihs, b * (cfg.blk // 16), cfg.blk, 2 * D)
                sel = pair_select(gpt, ipar[:, chsl], "sel")
                gb = gp_.tile([P, SB, D], bf16, tag="gb")
                emit_gather(gb, HdA, ihd, b * (cfg.blk // 16), cfg.blk, D)
                nc.vector.tensor_tensor(gb[:], sel[:], gb[:], Alu.mult)
                nc.vector.tensor_reduce(
                    alpha_i[:, chsl], gb[:], mybir.AxisListType.X, Alu.add
                )

            # ---------------- global softmax stats ----------------
            lred = cp.tile([P, 1], f32, tag="lred")
            pred = cp.tile([P, 1], f32, tag="pred")
            nc.vector.tensor_reduce(
                lred[:], alpha_i[:], mybir.AxisListType.X, Alu.max
            )
            nc.gpsimd.partition_all_reduce(
                pred[:], lred[:], channels=P, reduce_op=bass_isa.ReduceOp.max
            )
            cc_in = dr.tile([1, 1], f32, tag="cc_in")
            cc_out = dr.tile([1, 1], f32, tag="cc_out")
            nc.gpsimd.dma_start(out=cc_in[:], in_=pred[0:1, 0:1])
            nc.gpsimd.collective_compute(
                "AllReduce", Alu.max, replica_groups=[list(range(NC))],
                ins=[cc_in[:].opt()], outs=[cc_out[:].opt()],
            )
            gmax1 = cp.tile([1, 1], f32, tag="gmax1")
            nc.sync.dma_start(out=gmax1[:], in_=cc_out[:])
            negb = cp.tile([P, 1], f32, tag="negb")
            nc.gpsimd.partition_broadcast(negb[:], gmax1[:])
            nc.vector.tensor_scalar_mul(negb[:], negb[:], -SCALE)

            w_i = ip.tile([P, CH_I], f32, tag="w_i")
            nc.scalar.activation(
                w_i[:], alpha_i[:], Act.Exp, bias=negb[:], scale=SCALE
            )
            nc.vector.tensor_tensor(w_i[:], w_i[:], ival[:], Alu.mult)
            lsum = cp.tile([P, 1], f32, tag="lsum")
            psum_ = cp.tile([P, 1], f32, tag="psum_")
            nc.vector.tensor_reduce(
                lsum[:], w_i[:], mybir.AxisListType.X, Alu.add
            )
            nc.gpsimd.partition_all_reduce(
                psum_[:], lsum[:], channels=P, reduce_op=bass_isa.ReduceOp.add
            )
            z_in = dr.tile([1, 1], f32, tag="z_in")
            z_out = dr.tile([1, 1], f32, tag="z_out")
            nc.gpsimd.dma_start(out=z_in[:], in_=psum_[0:1, 0:1])
            nc.gpsimd.collective_compute(
                "AllReduce", Alu.add, replica_groups=[list(range(NC))],
                ins=[z_in[:].opt()], outs=[z_out[:].opt()],
            )
            zt = cp.tile([1, 1], f32, tag="zt")
            nc.sync.dma_start(out=zt[:], in_=z_out[:])
            invz1 = cp.tile([1, 1], f32, tag="invz1")
            nc.vector.reciprocal(invz1[:], zt[:])
            invz = cp.tile([P, 1], f32, tag="invz")
            nc.gpsimd.partition_broadcast(invz[:], invz1[:])
            nc.vector.tensor_tensor(
                w_i[:], w_i[:], invz[:].to_broadcast((P, CH_I)), Alu.mult
            )

            # ---------------- item-direction aggregation ----------------
            MAGIC = 12582912.0  # 1.5 * 2**23: add/sub forces RNE to integer

            def agg_blocks(nblk, cpt, w_src, dm_f, sel_fn, t_out, t_scale, tag_pfx):
                ps = None
                for b in range(nblk):
                    sel, wsl = sel_fn(b)
                    eqw = wp.tile([P, SB, P], bf16, tag=f"{tag_pfx}eqw")
                    nc.vector.tensor_tensor(
                        eqw[:],
                        dm_f[:, b * SB : (b + 1) * SB, None].to_broadcast((P, SB, P)),
                        iota_f[:, None, :].to_broadcast((P, SB, P)),
                        Alu.is_equal,
                    )
                    nc.vector.tensor_tensor(
                        eqw[:], eqw[:], wsl, Alu.mult
                    )
                    for sl in range(SB):
                        g = b * SB + sl
                        t, c = divmod(g, cpt)
                        if c == 0:
                            ps = pp.tile([P, D], f32, tag=f"{tag_pfx}ps")
                        nc.tensor.matmul(
                            out=ps[:], lhsT=eqw[:, sl, :], rhs=sel[:, sl, :],
                            start=(c == 0), stop=(c == cpt - 1),
                        )
                        if c == cpt - 1:
                            am = op_.tile([P, 1], f32, tag=f"{tag_pfx}am")
                            nc.vector.tensor_reduce(
                                am[:], ps[:], mybir.AxisListType.X, Alu.max,
                                apply_absolute_value=True,
                            )
                            nc.vector.tensor_scalar_add(am[:], am[:], 1e-30)
                            dsc = op_.tile([P, 1], f32, tag=f"{tag_pfx}dsc")
                            nc.vector.tensor_scalar_mul(dsc[:], am[:], 1.0 / 127.0)
                            nc.sync.dma_start(
                                out=t_scale[t * P : (t + 1) * P, :], in_=dsc[:]
                            )
                            k = op_.tile([P, 1], f32, tag=f"{tag_pfx}k")
                            nc.vector.reciprocal(k[:], am[:])
                            nc.vector.tensor_scalar_mul(k[:], k[:], 127.0)
                            sq = op_.tile([P, D], f32, tag=f"{tag_pfx}sq")
                            nc.scalar.activation(
                                sq[:], ps[:], Act.Copy, bias=MAGIC, scale=k[:]
                            )
                            nc.vector.tensor_scalar_sub(sq[:], sq[:], MAGIC)
                            ob = op_.tile([P, D], i8, tag=f"{tag_pfx}ob")
                            nc.vector.tensor_copy(ob[:], sq[:])
                            nc.sync.dma_start(
                                out=t_out[t * P : (t + 1) * P, :], in_=ob[:]
                            )

            def item_sel(b):
                i16sl = slice(b * (cfg.blk // 16), (b + 1) * (cfg.blk // 16))
                chsl = slice(b * SB, (b + 1) * SB)
                gpt = gp_.tile([P, SB, 2 * D], bf16, tag="gp")
                emit_gather(gpt, FsP, ihs, b * (cfg.blk // 16), cfg.blk, 2 * D)
                sel = pair_select(gpt, ipar[:, chsl], "sel")
                wsl = w_i[:, chsl, None].to_broadcast((P, SB, P))
                return sel, wsl

            agg_blocks(cfg.nblk_i, cfg.cpt_i, w_i, idmf, item_sel, t_io, t_is, "i")

            # ---------------- user-direction (alpha fused) ----------------
            w_ub = {}

            def user_sel(b):
                i16sl = slice(b * (cfg.blk // 16), (b + 1) * (cfg.blk // 16))
                chsl = slice(b * SB, (b + 1) * SB)
                gpt = gp_.tile([P, SB, 2 * D], bf16, tag="gp")
                emit_gather(gpt, HsP, uhs, b * (cfg.blk // 16), cfg.blk, 2 * D)
                sel = pair_select(gpt, upar[:, chsl], "sel")
                gb = gp_.tile([P, SB, D], bf16, tag="gb")
                emit_gather(gb, HdA, uhd, b * (cfg.blk // 16), cfg.blk, D)
                nc.vector.tensor_tensor(gb[:], sel[:], gb[:], Alu.mult)
                aub = wp.tile([P, SB], f32, tag="aub")
                nc.vector.tensor_reduce(
                    aub[:], gb[:], mybir.AxisListType.X, Alu.add
                )
                wub = wp.tile([P, SB], f32, tag="wub")
                nc.scalar.activation(
                    wub[:], aub[:], Act.Exp, bias=negb[:], scale=SCALE
                )
                nc.vector.tensor_tensor(
                    wub[:], wub[:], uval[:, chsl], Alu.mult
                )
                nc.vector.tensor_tensor(
                    wub[:], wub[:], invz[:].to_broadcast((P, SB)), Alu.mult
                )
                gfd = gp_.tile([P, SB, D], bf16, tag="gfd")
                emit_gather(gfd, FdA, uhd, b * (cfg.blk // 16), cfg.blk, D)
                wsl = wub[:, :, None].to_broadcast((P, SB, P))
                return gfd, wsl

            agg_blocks(cfg.nblk_u, cfg.cpt_u, None, udmf, user_sel, t_uo, t_us, "u")

    nc.finalize()
    return nc


# --------------------------------------------------------------------------
# host preprocessing
# --------------------------------------------------------------------------

def wrap16(a):
    """per-edge int array [NCORES, n] -> dma_gather 16-wrap layout [NC,16,n/16]."""
    ncore, n = a.shape
    return np.ascontiguousarray(
        a.reshape(ncore, n // 16, 16).transpose(0, 2, 1)
    ).astype(np.int16)


def glayout(a):
    """per-edge array [NCORES, n] -> gather-output layout [NC, 128, n/128]."""
    ncore, n = a.shape
    return np.ascontiguousarray(a.reshape(ncore, n // P, P).transpose(0, 2, 1))


def prep_direction(cfg: Cfg, own_ids, own_per_core, nt, cpt, hs_row, hd_row):
    """Sort/pad one direction's edges by (owner core, dest tile).

    own_ids: destination-side node ids (ownership + one-hot row)
    hs_row/hd_row: per-edge rows into the deduped gather tables
    """
    E = own_ids.shape[0]
    ec = nt * cpt * P
    own = own_ids // own_per_core
    loc = own_ids - own * own_per_core
    til = loc >> 7
    grp = own * nt + til
    order = np.argsort(grp, kind="stable")
    grp_s = grp[order]
    counts = np.bincount(grp, minlength=NC * nt)
    if counts.max() > cpt * P:
        return None
    starts = np.concatenate([[0], np.cumsum(counts)[:-1]])
    rank = np.arange(E, dtype=np.int64) - starts[grp_s]
    slot = (grp_s % nt) * (cpt * P) + rank
    core = grp_s // nt
    loc_s = loc[order]
    hs_s = hs_row[order]
    hd_s = hd_row[order]

    hs_idx = np.zeros((NC, ec), np.int32)
    hd_idx = np.zeros((NC, ec), np.int32)
    par = np.zeros((NC, ec), np.float32)
    val = np.zeros((NC, ec), np.float32)
    dmod = np.zeros((NC, ec), np.int32)
    hs_idx[core, slot] = hs_s >> 1
    par[core, slot] = hs_s & 1
    hd_idx[core, slot] = hd_s
    dmod[core, slot] = loc_s & 127
    val[core, slot] = 1.0
    return {
        "hs": wrap16(hs_idx),
        "hd": wrap16(hd_idx),
        "par": glayout(par).astype(BF16),
        "val": glayout(val).astype(BF16),
        "dm": glayout(dmod).astype(np.uint8),
    }


def host_prep_features(cfg: Cfg, feat, user_ids, item_ids):
    uu, inv_u = np.unique(user_ids, return_inverse=True)
    ii, inv_i = np.unique(item_ids, return_inverse=True)
    if len(uu) > NC * cfg.gtu or len(ii) > NC * cfg.gti:
        return None
    hs_sh = np.zeros((NC * cfg.gtu, D), BF16)
    hs_sh[: len(uu)] = feat[uu].astype(BF16)
    hd_sh = np.zeros((NC * cfg.gti, D), BF16)
    hd_sh[: len(ii)] = feat[ii].astype(BF16)
    return hs_sh, hd_sh, inv_u, inv_i


def host_prep_indices(cfg: Cfg, inv_u, inv_i, W_src, b_src, W_dst, b_dst,
                      edge_src, edge_dst):
    e_hs = inv_u[edge_src].astype(np.int64)
    e_hd = inv_i[edge_dst].astype(np.int64)

    idir = prep_direction(
        cfg, edge_dst, cfg.ipc, cfg.nt_i, cfg.cpt_i, e_hs, e_hd
    )
    udir = prep_direction(
        cfg, edge_src, cfg.upc, cfg.nt_u, cfg.cpt_u, e_hs, e_hd
    )
    if idir is None or udir is None:
        return None

    wst = np.ascontiguousarray(W_src.T).astype(BF16)
    wdt = np.ascontiguousarray(W_dst.T).astype(BF16)

    def rep(a):
        return np.broadcast_to(a, (NC, *a.shape))

    ins = {
        "wst": rep(wst), "wdt": rep(wdt),
        "bs": rep(b_src.reshape(1, D).astype(np.float32)),
        "bd": rep(b_dst.reshape(1, D).astype(np.float32)),
        "ihs": idir["hs"], "ihd": idir["hd"], "ipar": idir["par"],
        "ival": idir["val"], "idm": idir["dm"],
        "uhs": udir["hs"], "uhd": udir["hd"], "upar": udir["par"],
        "uval": udir["val"], "udm": udir["dm"],
    }
    return {k: np.ascontiguousarray(v.reshape(-1, *v.shape[2:])) for k, v in ins.items()}


def host_prep(cfg: Cfg, feat, user_ids, item_ids, W_src, b_src, W_dst, b_dst,
              edge_src, edge_dst):
    """Non-overlapped variant (used by the sim tests)."""
    f = host_prep_features(cfg, feat, user_ids, item_ids)
    if f is None:
        return None
    hs_sh, hd_sh, inv_u, inv_i = f
    ins = host_prep_indices(
        cfg, inv_u, inv_i, W_src, b_src, W_dst, b_dst, edge_src, edge_dst
    )
    if ins is None:
        return None
    ins["hs"] = hs_sh
    ins["hd"] = hd_sh
    return ins


# --------------------------------------------------------------------------
# cached SPMD runner (jit built once, zeros created on-device)
# --------------------------------------------------------------------------

class SpmdRunner:
    def __init__(self, nc, n_cores):
        import jax
        import jax.numpy as jnp
        from jax.sharding import Mesh, NamedSharding, PartitionSpec
        from jax.experimental.shard_map import shard_map
        from concourse import mybir
        from concourse.bass2jax import (
            _bass_exec_p, partition_id_tensor, install_neuronx_cc_hook,
        )

        install_neuronx_cc_hook()
        partition_name = (
            nc.partition_id_tensor.name if nc.partition_id_tensor else None
        )
        in_names, out_names, out_avals, zero_shapes = [], [], [], []
        for alloc in nc.m.functions[0].allocations:
            if not isinstance(alloc, mybir.MemoryLocationSet):
                continue
            name = alloc.memorylocations[0].name
            if alloc.kind == "ExternalInput":
                if name != partition_name:
                    in_names.append(name)
            elif alloc.kind == "ExternalOutput":
                out_names.append(name)
                shape = tuple(alloc.tensor_shape)
                dtype = mybir.dt.np(alloc.dtype)
                out_avals.append(jax.core.ShapedArray(shape, dtype))
                zero_shapes.append((shape, dtype))
        self.in_names = in_names
        self.out_names = out_names
        n_params = len(in_names)
        n_outs = len(out_avals)
        all_in = list(in_names) + list(out_names)
        if partition_name is not None:
            all_in.append(partition_name)
        donate = tuple(range(n_params, n_params + n_outs))

        def _body(*args):
            operands = list(args)
            if partition_name is not None:
                operands.append(partition_id_tensor())
            outs = _bass_exec_p.bind(
                *operands,
                out_avals=tuple(out_avals),
                in_names=tuple(all_in),
                out_names=tuple(out_names),
                lowering_input_output_aliases=(),
                sim_require_finite=False,
                sim_require_nnan=False,
                nc=nc,
            )
            return tuple(outs)

        devices = jax.devices()[:n_cores]
        mesh = Mesh(np.asarray(devices), ("core",))
        in_specs = (PartitionSpec("core"),) * (n_params + n_outs)
        out_specs = (PartitionSpec("core"),) * n_outs
        self.sharded = jax.jit(
            shard_map(
                _body, mesh=mesh, in_specs=in_specs, out_specs=out_specs,
                check_rep=False,
            ),
            donate_argnums=donate,
            keep_unused=True,
        )
        shd = NamedSharding(mesh, PartitionSpec("core"))
        self.zeros_fn = jax.jit(
            lambda: tuple(
                jnp.zeros((n_cores * s[0], *s[1:]), d) for s, d in zero_shapes
            ),
            out_shardings=(shd,) * n_outs,
        )

        self._shd = shd

    def put(self, arr):
        import jax

        return jax.device_put(arr, self._shd)

    def __call__(self, stacked_inputs):
        args = [stacked_inputs[n] for n in self.in_names]
        zeros = self.zeros_fn()
        outs = self.sharded(*args, *zeros)
        return dict(zip(self.out_names, outs))


# --------------------------------------------------------------------------
# fallback: pure-host computation (only if capacity asserts fail)
# --------------------------------------------------------------------------

def _host_fallback(feat, W_src, b_src, W_dst, b_dst, user_ids, item_ids,
                   edge_src, edge_dst):
    H_src = feat[user_ids]
    H_dst = feat[item_ids]
    alpha = np.einsum(
        "ed,ed->e", H_src[edge_src], H_dst[edge_dst], optimize=True
    ) * SCALE
    w = np.exp(alpha - alpha.max())
    w /= w.sum()
    FS = np.maximum(H_src @ W_src.T + b_src[None, :], 0.0)
    FD = np.maximum(H_dst @ W_dst.T + b_dst[None, :], 0.0)

    def seg_sum(vals, seg, nseg):
        out = np.zeros((nseg, D), np.float32)
        np.add.at(out, seg, vals)
        return out

    item_new = seg_sum(FS[edge_src] * w[:, None], edge_dst, len(item_ids))
    user_new = seg_sum(FD[edge_dst] * w[:, None], edge_src, len(user_ids))
    return np.concatenate([user_new, item_new], 0).astype(np.float32)


# --------------------------------------------------------------------------
# entry point
# --------------------------------------------------------------------------

def kernel(**inputs):
    import threading

    cfg = FULL
    feat = np.asarray(inputs["feat"], np.float32)
    W_src = np.asarray(inputs["W_src"], np.float32)
    b_src = np.asarray(inputs["b_src"], np.float32)
    W_dst = np.asarray(inputs["W_dst"], np.float32)
    b_dst = np.asarray(inputs["b_dst"], np.float32)
    user_ids = np.asarray(inputs["user_ids"]).astype(np.int64)
    item_ids = np.asarray(inputs["item_ids"]).astype(np.int64)
    edge_src = np.asarray(inputs["edge_src"]).astype(np.int64)
    edge_dst = np.asarray(inputs["edge_dst"]).astype(np.int64)

    f = host_prep_features(cfg, feat, user_ids, item_ids)
    if f is None:
        return _host_fallback(
            feat, W_src, b_src, W_dst, b_dst, user_ids, item_ids,
            edge_src, edge_dst,
        )
    hs_sh, hd_sh, inv_u, inv_i = f

    if "runner" not in _compiled:
        nc = build(cfg)
        _compiled["runner"] = SpmdRunner(nc, NC)
    runner = _compiled["runner"]

    # upload the big feature tables while the host builds edge indices
    staged = {}

    def _stage():
        staged["hs"] = runner.put(hs_sh)
        staged["hd"] = runner.put(hd_sh)

    th = threading.Thread(target=_stage)
    th.start()
    ins = host_prep_indices(
        cfg, inv_u, inv_i, W_src, b_src, W_dst, b_dst, edge_src, edge_dst
    )
    th.join()
    if ins is None:
        return _host_fallback(
            feat, W_src, b_src, W_dst, b_dst, user_ids, item_ids,
            edge_src, edge_dst,
        )
    ins["hs"] = staged["hs"]
    ins["hd"] = staged["hd"]

    outs = runner(ins)
    LAST["results"] = None

    fetched = {}

    def _fetch_small():
        fetched["io"] = np.asarray(outs["io"])
        fetched["us"] = np.asarray(outs["us"])
        fetched["is_"] = np.asarray(outs["is_"])

    th2 = threading.Thread(target=_fetch_small)
    th2.start()
    uo = np.asarray(outs["uo"]).reshape(NC, cfg.nt_u * P, D)[:, : cfg.upc]
    th2.join()
    us = fetched["us"].reshape(NC, cfg.nt_u * P, 1)[:, : cfg.upc]
    io = fetched["io"].reshape(NC, cfg.nt_i * P, D)[:, : cfg.ipc]
    is_ = fetched["is_"].reshape(NC, cfg.nt_i * P, 1)[:, : cfg.ipc]
    user_new = uo.astype(np.float32) * us
    item_new = io.astype(np.float32) * is_
    return np.concatenate(
        [user_new.reshape(-1, D), item_new.reshape(-1, D)], 0
    )


# revision 10
# speedup vs baseline: 8.2015x; 1.0149x over previous
"""Trainium2 Bass kernel for the bipartite GNN message-passing layer.

All compute runs on the 8 NeuronCores:
  - node features are uploaded row-sharded in bf16 and AllGathered on-device
  - dense transforms (relu(H @ W^T + b)) run row-sharded on the PE array
  - edge dot-product attention (global softmax) and the alpha-weighted
    segment sums run edge-sharded: each core owns the edges whose
    destination (resp. source) it owns, so aggregation needs no collective
  - segment sums are computed as one-hot matmuls accumulated in PSUM over
    destination tiles (host pre-sorts edges by destination tile with fixed
    per-tile capacity), so no scatter-add races
  - outputs leave the device in bf16 to halve D2H traffic

Host does only O(E) integer index preprocessing and the initial
feat[user_ids]/feat[item_ids] row gathers.
"""
import os
import sys
from dataclasses import dataclass

import numpy as np

for _p in ("/opt/trn_rl_repo",):
    if _p not in sys.path and os.path.isdir(_p):
        sys.path.insert(0, _p)

import ml_dtypes

BF16 = ml_dtypes.bfloat16
P = 128
D = 256
NC = 8
SCALE = 1.0 / 16.0


@dataclass(frozen=True)
class Cfg:
    n_nodes: int
    upc: int      # users per core
    upadc: int    # padded users per core (multiple of 128)
    ipc: int      # items per core
    ipadc: int    # padded items per core (multiple of 128)
    nt_i: int     # item output tiles per core
    cpt_i: int    # chunks (128 edges) per item tile
    nt_u: int     # user output tiles per core (>= ceil(upc/128))
    cpt_u: int
    blk: int      # edges per gather block (multiple of 128)
    gtu: int      # deduped user-node gather-table rows per core (mult of 128)
    gti: int      # deduped item-node gather-table rows per core (mult of 128)

    @property
    def ec_i(self):
        return self.nt_i * self.cpt_i * P

    @property
    def ec_u(self):
        return self.nt_u * self.cpt_u * P

    @property
    def nblk_i(self):
        return self.ec_i // self.blk

    @property
    def nblk_u(self):
        return self.ec_u // self.blk

    @property
    def sb(self):
        return self.blk // P


FULL = Cfg(
    n_nodes=70000,
    upc=6250, upadc=6272, ipc=2500, ipadc=2560,
    nt_i=20, cpt_i=20, nt_u=50, cpt_u=8, blk=2048,
    gtu=4608, gti=2304,
)

_compiled = {}
LAST = {}


# --------------------------------------------------------------------------
# device kernel
# --------------------------------------------------------------------------

def build(cfg: Cfg):
    import concourse.bacc as bacc
    import concourse.mybir as mybir
    import concourse.tile as tile
    import concourse.bass_isa as bass_isa

    f32 = mybir.dt.float32
    bf16 = mybir.dt.bfloat16
    i16 = mybir.dt.int16
    i32 = mybir.dt.int32
    u8 = mybir.dt.uint8
    Alu = mybir.AluOpType
    Act = mybir.ActivationFunctionType
    SB = cfg.sb

    nc = bacc.Bacc("TRN2", target_bir_lowering=False, debug=False, num_devices=NC)

    t_hs = nc.dram_tensor("hs", [cfg.gtu, D], bf16, kind="ExternalInput")
    t_hd = nc.dram_tensor("hd", [cfg.gti, D], bf16, kind="ExternalInput")
    t_wst = nc.dram_tensor("wst", [2 * P, D], bf16, kind="ExternalInput")
    t_wdt = nc.dram_tensor("wdt", [2 * P, D], bf16, kind="ExternalInput")
    t_bs = nc.dram_tensor("bs", [1, D], f32, kind="ExternalInput")
    t_bd = nc.dram_tensor("bd", [1, D], f32, kind="ExternalInput")
    # item-direction (edges sharded by destination-item owner, sorted by tile)
    t_ihs = nc.dram_tensor("ihs", [16, cfg.ec_i // 16], i16, kind="ExternalInput")
    t_ihd = nc.dram_tensor("ihd", [16, cfg.ec_i // 16], i16, kind="ExternalInput")
    t_ipar = nc.dram_tensor("ipar", [P, cfg.ec_i // P], bf16, kind="ExternalInput")
    t_ival = nc.dram_tensor("ival", [P, cfg.ec_i // P], bf16, kind="ExternalInput")
    t_idm = nc.dram_tensor("idm", [P, cfg.ec_i // P], u8, kind="ExternalInput")
    # user-direction
    t_uhs = nc.dram_tensor("uhs", [16, cfg.ec_u // 16], i16, kind="ExternalInput")
    t_uhd = nc.dram_tensor("uhd", [16, cfg.ec_u // 16], i16, kind="ExternalInput")
    t_upar = nc.dram_tensor("upar", [P, cfg.ec_u // P], bf16, kind="ExternalInput")
    t_uval = nc.dram_tensor("uval", [P, cfg.ec_u // P], bf16, kind="ExternalInput")
    t_udm = nc.dram_tensor("udm", [P, cfg.ec_u // P], u8, kind="ExternalInput")

    i8 = mybir.dt.int8
    t_uo = nc.dram_tensor("uo", [cfg.nt_u * P, D], i8, kind="ExternalOutput")
    t_io = nc.dram_tensor("io", [cfg.nt_i * P, D], i8, kind="ExternalOutput")
    t_us = nc.dram_tensor("us", [cfg.nt_u * P, 1], f32, kind="ExternalOutput")
    t_is = nc.dram_tensor("is_", [cfg.nt_i * P, 1], f32, kind="ExternalOutput")

    CH_I = cfg.nt_i * cfg.cpt_i
    CH_U = cfg.nt_u * cfg.cpt_u

    with tile.TileContext(nc) as tc:
        with (
            tc.tile_pool(name="const", bufs=1) as cp,
            tc.tile_pool(name="idx", bufs=1) as ip,
            tc.tile_pool(name="dram", bufs=1, space="DRAM") as dr,
            tc.tile_pool(name="ps", bufs=2, space="PSUM") as pp,
            tc.tile_pool(name="gth", bufs=1) as gp_,
            tc.tile_pool(name="wrk", bufs=1) as wp,
            tc.tile_pool(name="out", bufs=2) as op_,
        ):
            # ---------------- constants / index staging ----------------
            iota_i = cp.tile([P, P], i32, tag="iota_i")
            nc.gpsimd.iota(iota_i[:], pattern=[[1, P]], base=0, channel_multiplier=0)
            iota_f = cp.tile([P, P], f32, tag="iota_f")
            nc.vector.tensor_copy(iota_f[:], iota_i[:])

            wst0 = cp.tile([P, D], bf16, tag="wst0")
            wst1 = cp.tile([P, D], bf16, tag="wst1")
            wdt0 = cp.tile([P, D], bf16, tag="wdt0")
            wdt1 = cp.tile([P, D], bf16, tag="wdt1")
            nc.sync.dma_start(out=wst0[:], in_=t_wst[0:P, :])
            nc.sync.dma_start(out=wst1[:], in_=t_wst[P : 2 * P, :])
            nc.sync.dma_start(out=wdt0[:], in_=t_wdt[0:P, :])
            nc.sync.dma_start(out=wdt1[:], in_=t_wdt[P : 2 * P, :])

            bias = {}
            for key, tb in (("s", t_bs), ("d", t_bd)):
                b1 = cp.tile([1, D], f32, tag=f"b1{key}")
                nc.sync.dma_start(out=b1[:], in_=tb[:])
                bb = cp.tile([P, D], f32, tag=f"bb{key}")
                nc.gpsimd.partition_broadcast(bb[:], b1[:])
                bias[key] = bb

            def stage_idx(th, n, tag):
                t = ip.tile([P, n // 16], i16, tag=tag)
                for k in range(8):
                    nc.sync.dma_start(out=t[16 * k : 16 * (k + 1), :], in_=th[:, :])
                return t

            ihs = stage_idx(t_ihs, cfg.ec_i, "ihs")
            ihd = stage_idx(t_ihd, cfg.ec_i, "ihd")
            uhs = stage_idx(t_uhs, cfg.ec_u, "uhs")
            uhd = stage_idx(t_uhd, cfg.ec_u, "uhd")

            def stage_pl(th, n, dt, tag):
                t = ip.tile([P, n // P], dt, tag=tag)
                nc.sync.dma_start(out=t[:], in_=th[:, :])
                return t

            ipar = stage_pl(t_ipar, cfg.ec_i, bf16, "ipar")
            ival = stage_pl(t_ival, cfg.ec_i, bf16, "ival")
            idm8 = stage_pl(t_idm, cfg.ec_i, u8, "idm8")
            upar = stage_pl(t_upar, cfg.ec_u, bf16, "upar")
            uval = stage_pl(t_uval, cfg.ec_u, bf16, "uval")
            udm8 = stage_pl(t_udm, cfg.ec_u, u8, "udm8")

            idmf = ip.tile([P, CH_I], f32, tag="idmf")
            nc.vector.tensor_copy(idmf[:], idm8[:])
            udmf = ip.tile([P, CH_U], f32, tag="udmf")
            nc.vector.tensor_copy(udmf[:], udm8[:])

            # ---------------- AllGather H tables ----------------
            hs_b = dr.tile([cfg.gtu, D], bf16, tag="hs_b")
            hd_b = dr.tile([cfg.gti, D], bf16, tag="hd_b")
            nc.gpsimd.dma_start(out=hs_b[:], in_=t_hs[:])
            nc.gpsimd.dma_start(out=hd_b[:], in_=t_hd[:])
            HsF = dr.tile([NC * cfg.gtu, D], bf16, tag="HsF")
            HdF = dr.tile([NC * cfg.gti, D], bf16, tag="HdF")
            nc.gpsimd.collective_compute(
                "AllGather", Alu.bypass, replica_groups=[list(range(NC))],
                ins=[hs_b[:].opt()], outs=[HsF[:].opt()],
            )
            nc.gpsimd.collective_compute(
                "AllGather", Alu.bypass, replica_groups=[list(range(NC))],
                ins=[hd_b[:].opt()], outs=[HdF[:].opt()],
            )
            HsP = HsF[:].rearrange("(a b) c -> a (b c)", b=2)
            HdA = HdF[:]

            # ---------------- dense transforms ----------------
            fs_sh = dr.tile([cfg.gtu, D], bf16, tag="fs_sh")
            fd_sh = dr.tile([cfg.gti, D], bf16, tag="fd_sh")

            for key, t_in, npad, w0, w1, f_out in (
                ("s", t_hs, cfg.gtu, wst0, wst1, fs_sh),
                ("d", t_hd, cfg.gti, wdt0, wdt1, fd_sh),
            ):
                htA = cp.tile([P, npad], bf16, tag=f"htA{key}")
                htB = cp.tile([P, npad], bf16, tag=f"htB{key}")
                nc.sync.dma_start_transpose(htA[:], t_in[:, 0:P])
                nc.sync.dma_start_transpose(htB[:], t_in[:, P : 2 * P])
                for t in range(npad // P):
                    ps = pp.tile([P, D], f32, tag="mmps")
                    nc.tensor.matmul(
                        out=ps[:], lhsT=htA[:, t * P : (t + 1) * P], rhs=w0[:],
                        start=True, stop=False,
                    )
                    nc.tensor.matmul(
                        out=ps[:], lhsT=htB[:, t * P : (t + 1) * P], rhs=w1[:],
                        start=False, stop=True,
                    )
                    tmp = op_.tile([P, D], f32, tag="mmtmp")
                    nc.vector.tensor_tensor(tmp[:], ps[:], bias[key][:], Alu.add)
                    ft = op_.tile([P, D], bf16, tag="mmft")
                    nc.vector.tensor_scalar_max(ft[:], tmp[:], 0.0)
                    nc.sync.dma_start(out=f_out[t * P : (t + 1) * P, :], in_=ft[:])

            FsF = dr.tile([NC * cfg.gtu, D], bf16, tag="FsF")
            FdF = dr.tile([NC * cfg.gti, D], bf16, tag="FdF")
            nc.gpsimd.collective_compute(
                "AllGather", Alu.bypass, replica_groups=[list(range(NC))],
                ins=[fs_sh[:].opt()], outs=[FsF[:].opt()],
            )
            nc.gpsimd.collective_compute(
                "AllGather", Alu.bypass, replica_groups=[list(range(NC))],
                ins=[fd_sh[:].opt()], outs=[FdF[:].opt()],
            )
            FsP = FsF[:].rearrange("(a b) c -> a (b c)", b=2)
            FdA = FdF[:]

            # ---------------- item-direction alphas ----------------
            alpha_i = ip.tile([P, CH_I], f32, tag="alpha_i")

            GCALL = 1024  # max idxs per dma_gather (16-DMA ring: 128 descs)

            def emit_gather(dst, table, idxt, base_col, blk, elem):
                n = 0
                while n < blk:
                    step = min(GCALL, blk - n)
                    nc.gpsimd.dma_gather(
                        dst[:, n // P : (n + step) // P, :],
                        table,
                        idxt[:, base_col + n // 16 : base_col + (n + step) // 16],
                        step, step, elem,
                    )
                    n += step

            def pair_select(gpt, par_sl, tag):
                """gpt [P,SB,2D] pair-gather; returns selected [P,SB,D] bf16."""
                sel = wp.tile([P, SB, D], bf16, tag=tag)
                nc.vector.tensor_tensor(
                    sel[:], gpt[:, :, D : 2 * D], gpt[:, :, 0:D], Alu.subtract
                )
                nc.vector.tensor_tensor(
                    sel[:], sel[:],
                    par_sl[:, :, None].to_broadcast((P, SB, D)), Alu.mult,
                )
                nc.vector.tensor_tensor(sel[:], sel[:], gpt[:, :, 0:D], Alu.add)
                return sel

            for b in range(cfg.nblk_i):
                i16sl = slice(b * (cfg.blk // 16), (b + 1) * (cfg.blk // 16))
                chsl = slice(b * SB, (b + 1) * SB)
                gpt = gp_.tile([P, SB, 2 * D], bf16, tag="gp")
                emit_gather(gpt, HsP, ihs, b * (cfg.blk // 16), cfg.blk, 2 * D)
                sel = pair_select(gpt, ipar[:, chsl], "sel")
                gb = gp_.tile([P, SB, D], bf16, tag="gb")
                emit_gather(gb, HdA, ihd, b * (cfg.blk // 16), cfg.blk, D)
                nc.vector.tensor_tensor(gb[:], sel[:], gb[:], Alu.mult)
                nc.vector.tensor_reduce(
                    alpha_i[:, chsl], gb[:], mybir.AxisListType.X, Alu.add
                )

            # ---------------- global softmax stats ----------------
            lred = cp.tile([P, 1], f32, tag="lred")
            pred = cp.tile([P, 1], f32, tag="pred")
            nc.vector.tensor_reduce(
                lred[:], alpha_i[:], mybir.AxisListType.X, Alu.max
            )
            nc.gpsimd.partition_all_reduce(
                pred[:], lred[:], channels=P, reduce_op=bass_isa.ReduceOp.max
            )
            cc_in = dr.tile([1, 1], f32, tag="cc_in")
            cc_out = dr.tile([1, 1], f32, tag="cc_out")
            nc.gpsimd.dma_start(out=cc_in[:], in_=pred[0:1, 0:1])
            nc.gpsimd.collective_compute(
                "AllReduce", Alu.max, replica_groups=[list(range(NC))],
                ins=[cc_in[:].opt()], outs=[cc_out[:].opt()],
            )
            gmax1 = cp.tile([1, 1], f32, tag="gmax1")
            nc.sync.dma_start(out=gmax1[:], in_=cc_out[:])
            negb = cp.tile([P, 1], f32, tag="negb")
            nc.gpsimd.partition_broadcast(negb[:], gmax1[:])
            nc.vector.tensor_scalar_mul(negb[:], negb[:], -SCALE)

            w_i = ip.tile([P, CH_I], f32, tag="w_i")
            nc.scalar.activation(
                w_i[:], alpha_i[:], Act.Exp, bias=negb[:], scale=SCALE
            )
            nc.vector.tensor_tensor(w_i[:], w_i[:], ival[:], Alu.mult)
            lsum = cp.tile([P, 1], f32, tag="lsum")
            psum_ = cp.tile([P, 1], f32, tag="psum_")
            nc.vector.tensor_reduce(
                lsum[:], w_i[:], mybir.AxisListType.X, Alu.add
            )
            nc.gpsimd.partition_all_reduce(
                psum_[:], lsum[:], channels=P, reduce_op=bass_isa.ReduceOp.add
            )
            z_in = dr.tile([1, 1], f32, tag="z_in")
            z_out = dr.tile([1, 1], f32, tag="z_out")
            nc.gpsimd.dma_start(out=z_in[:], in_=psum_[0:1, 0:1])
            nc.gpsimd.collective_compute(
                "AllReduce", Alu.add, replica_groups=[list(range(NC))],
                ins=[z_in[:].opt()], outs=[z_out[:].opt()],
            )
            zt = cp.tile([1, 1], f32, tag="zt")
            nc.sync.dma_start(out=zt[:], in_=z_out[:])
            invz1 = cp.tile([1, 1], f32, tag="invz1")
            nc.vector.reciprocal(invz1[:], zt[:])
            invz = cp.tile([P, 1], f32, tag="invz")
            nc.gpsimd.partition_broadcast(invz[:], invz1[:])
            nc.vector.tensor_tensor(
                w_i[:], w_i[:], invz[:].to_broadcast((P, CH_I)), Alu.mult
            )

            # ---------------- item-direction aggregation ----------------
            MAGIC = 12582912.0  # 1.5 * 2**23: add/sub forces RNE to integer

            def agg_blocks(nblk, cpt, w_src, dm_f, sel_fn, t_out, t_scale, tag_pfx):
                ps = None
                for b in range(nblk):
                    sel, wsl = sel_fn(b)
                    eqw = wp.tile([P, SB, P], bf16, tag=f"{tag_pfx}eqw")
                    nc.vector.tensor_tensor(
                        eqw[:],
                        dm_f[:, b * SB : (b + 1) * SB, None].to_broadcast((P, SB, P)),
                        iota_f[:, None, :].to_broadcast((P, SB, P)),
                        Alu.is_equal,
                    )
                    nc.vector.tensor_tensor(
                        eqw[:], eqw[:], wsl, Alu.mult
                    )
                    for sl in range(SB):
                        g = b * SB + sl
                        t, c = divmod(g, cpt)
                        if c == 0:
                            ps = pp.tile([P, D], f32, tag=f"{tag_pfx}ps")
                        nc.tensor.matmul(
                            out=ps[:], lhsT=eqw[:, sl, :], rhs=sel[:, sl, :],
                            start=(c == 0), stop=(c == cpt - 1),
                        )
                        if c == cpt - 1:
                            am = op_.tile([P, 1], f32, tag=f"{tag_pfx}am")
                            nc.vector.tensor_reduce(
                                am[:], ps[:], mybir.AxisListType.X, Alu.max,
                                apply_absolute_value=True,
                            )
                            nc.vector.tensor_scalar_add(am[:], am[:], 1e-30)
                            dsc = op_.tile([P, 1], f32, tag=f"{tag_pfx}dsc")
                            nc.vector.tensor_scalar_mul(dsc[:], am[:], 1.0 / 127.0)
                            nc.sync.dma_start(
                                out=t_scale[t * P : (t + 1) * P, :], in_=dsc[:]
                            )
                            k = op_.tile([P, 1], f32, tag=f"{tag_pfx}k")
                            nc.vector.reciprocal(k[:], am[:])
                            nc.vector.tensor_scalar_mul(k[:], k[:], 127.0)
                            sq = op_.tile([P, D], f32, tag=f"{tag_pfx}sq")
                            nc.scalar.activation(
                                sq[:], ps[:], Act.Copy, bias=MAGIC, scale=k[:]
                            )
                            nc.vector.tensor_scalar_sub(sq[:], sq[:], MAGIC)
                            ob = op_.tile([P, D], i8, tag=f"{tag_pfx}ob")
                            nc.vector.tensor_copy(ob[:], sq[:])
                            nc.sync.dma_start(
                                out=t_out[t * P : (t + 1) * P, :], in_=ob[:]
                            )

            def item_sel(b):
                i16sl = slice(b * (cfg.blk // 16), (b + 1) * (cfg.blk // 16))
                chsl = slice(b * SB, (b + 1) * SB)
                gpt = gp_.tile([P, SB, 2 * D], bf16, tag="gp")
                emit_gather(gpt, FsP, ihs, b * (cfg.blk // 16), cfg.blk, 2 * D)
                sel = pair_select(gpt, ipar[:, chsl], "sel")
                wsl = w_i[:, chsl, None].to_broadcast((P, SB, P))
                return sel, wsl

            agg_blocks(cfg.nblk_i, cfg.cpt_i, w_i, idmf, item_sel, t_io, t_is, "i")

            # ---------------- user-direction (alpha fused) ----------------
            w_ub = {}

            def user_sel(b):
                i16sl = slice(b * (cfg.blk // 16), (b + 1) * (cfg.blk // 16))
                chsl = slice(b * SB, (b + 1) * SB)
                gpt = gp_.tile([P, SB, 2 * D], bf16, tag="gp")
                emit_gather(gpt, HsP, uhs, b * (cfg.blk // 16), cfg.blk, 2 * D)
                sel = pair_select(gpt, upar[:, chsl], "sel")
                gb = gp_.tile([P, SB, D], bf16, tag="gb")
                emit_gather(gb, HdA, uhd, b * (cfg.blk // 16), cfg.blk, D)
                nc.vector.tensor_tensor(gb[:], sel[:], gb[:], Alu.mult)
                aub = wp.tile([P, SB], f32, tag="aub")
                nc.vector.tensor_reduce(
                    aub[:], gb[:], mybir.AxisListType.X, Alu.add
                )
                wub = wp.tile([P, SB], f32, tag="wub")
                nc.scalar.activation(
                    wub[:], aub[:], Act.Exp, bias=negb[:], scale=SCALE
                )
                nc.vector.tensor_tensor(
                    wub[:], wub[:], uval[:, chsl], Alu.mult
                )
                nc.vector.tensor_tensor(
                    wub[:], wub[:], invz[:].to_broadcast((P, SB)), Alu.mult
                )
                gfd = gp_.tile([P, SB, D], bf16, tag="gfd")
                emit_gather(gfd, FdA, uhd, b * (cfg.blk // 16), cfg.blk, D)
                wsl = wub[:, :, None].to_broadcast((P, SB, P))
                return gfd, wsl

            agg_blocks(cfg.nblk_u, cfg.cpt_u, None, udmf, user_sel, t_uo, t_us, "u")

    nc.finalize()
    return nc


# --------------------------------------------------------------------------
# host preprocessing
# --------------------------------------------------------------------------

def wrap16(a):
    """per-edge int array [NCORES, n] -> dma_gather 16-wrap layout [NC,16,n/16]."""
    ncore, n = a.shape
    return np.ascontiguousarray(
        a.reshape(ncore, n // 16, 16).transpose(0, 2, 1)
    ).astype(np.int16)


def glayout(a):
    """per-edge array [NCORES, n] -> gather-output layout [NC, 128, n/128]."""
    ncore, n = a.shape
    return np.ascontiguousarray(a.reshape(ncore, n // P, P).transpose(0, 2, 1))


def prep_direction(cfg: Cfg, own_ids, own_per_core, nt, cpt, hs_row, hd_row):
    """Sort/pad one direction's edges by (owner core, dest tile).

    own_ids: destination-side node ids (ownership + one-hot row)
    hs_row/hd_row: per-edge rows into the deduped gather tables
    """
    E = own_ids.shape[0]
    ec = nt * cpt * P
    own = own_ids // own_per_core
    loc = own_ids - own * own_per_core
    til = loc >> 7
    grp = own * nt + til
    order = np.argsort(grp, kind="stable")
    grp_s = grp[order]
    counts = np.bincount(grp, minlength=NC * nt)
    if counts.max() > cpt * P:
        return None
    starts = np.concatenate([[0], np.cumsum(counts)[:-1]])
    rank = np.arange(E, dtype=np.int64) - starts[grp_s]
    slot = (grp_s % nt) * (cpt * P) + rank
    core = grp_s // nt
    loc_s = loc[order]
    hs_s = hs_row[order]
    hd_s = hd_row[order]

    hs_idx = np.zeros((NC, ec), np.int32)
    hd_idx = np.zeros((NC, ec), np.int32)
    par = np.zeros((NC, ec), np.float32)
    val = np.zeros((NC, ec), np.float32)
    dmod = np.zeros((NC, ec), np.int32)
    hs_idx[core, slot] = hs_s >> 1
    par[core, slot] = hs_s & 1
    hd_idx[core, slot] = hd_s
    dmod[core, slot] = loc_s & 127
    val[core, slot] = 1.0
    return {
        "hs": wrap16(hs_idx),
        "hd": wrap16(hd_idx),
        "par": glayout(par).astype(BF16),
        "val": glayout(val).astype(BF16),
        "dm": glayout(dmod).astype(np.uint8),
    }


def host_prep_features(cfg: Cfg, feat, user_ids, item_ids):
    uu, inv_u = np.unique(user_ids, return_inverse=True)
    ii, inv_i = np.unique(item_ids, return_inverse=True)
    if len(uu) > NC * cfg.gtu or len(ii) > NC * cfg.gti:
        return None
    hs_sh = np.zeros((NC * cfg.gtu, D), BF16)
    hs_sh[: len(uu)] = feat[uu].astype(BF16)
    hd_sh = np.zeros((NC * cfg.gti, D), BF16)
    hd_sh[: len(ii)] = feat[ii].astype(BF16)
    return hs_sh, hd_sh, inv_u, inv_i


def host_prep_indices(cfg: Cfg, inv_u, inv_i, W_src, b_src, W_dst, b_dst,
                      edge_src, edge_dst):
    e_hs = inv_u[edge_src].astype(np.int64)
    e_hd = inv_i[edge_dst].astype(np.int64)

    idir = prep_direction(
        cfg, edge_dst, cfg.ipc, cfg.nt_i, cfg.cpt_i, e_hs, e_hd
    )
    udir = prep_direction(
        cfg, edge_src, cfg.upc, cfg.nt_u, cfg.cpt_u, e_hs, e_hd
    )
    if idir is None or udir is None:
        return None

    wst = np.ascontiguousarray(W_src.T).astype(BF16)
    wdt = np.ascontiguousarray(W_dst.T).astype(BF16)

    def rep(a):
        return np.broadcast_to(a, (NC, *a.shape))

    ins = {
        "wst": rep(wst), "wdt": rep(wdt),
        "bs": rep(b_src.reshape(1, D).astype(np.float32)),
        "bd": rep(b_dst.reshape(1, D).astype(np.float32)),
        "ihs": idir["hs"], "ihd": idir["hd"], "ipar": idir["par"],
        "ival": idir["val"], "idm": idir["dm"],
        "uhs": udir["hs"], "uhd": udir["hd"], "upar": udir["par"],
        "uval": udir["val"], "udm": udir["dm"],
    }
    return {k: np.ascontiguousarray(v.reshape(-1, *v.shape[2:])) for k, v in ins.items()}


def host_prep(cfg: Cfg, feat, user_ids, item_ids, W_src, b_src, W_dst, b_dst,
              edge_src, edge_dst):
    """Non-overlapped variant (used by the sim tests)."""
    f = host_prep_features(cfg, feat, user_ids, item_ids)
    if f is None:
        return None
    hs_sh, hd_sh, inv_u, inv_i = f
    ins = host_prep_indices(
        cfg, inv_u, inv_i, W_src, b_src, W_dst, b_dst, edge_src, edge_dst
    )
    if ins is None:
        return None
    ins["hs"] = hs_sh
    ins["hd"] = hd_sh
    return ins


# --------------------------------------------------------------------------
# cached SPMD runner (jit built once, zeros created on-device)
# --------------------------------------------------------------------------

class SpmdRunner:
    def __init__(self, nc, n_cores):
        import jax
        import jax.numpy as jnp
        from jax.sharding import Mesh, NamedSharding, PartitionSpec
        from jax.experimental.shard_map import shard_map
        from concourse import mybir
        from concourse.bass2jax import (
            _bass_exec_p, partition_id_tensor, install_neuronx_cc_hook,
        )

        install_neuronx_cc_hook()
        partition_name = (
            nc.partition_id_tensor.name if nc.partition_id_tensor else None
        )
        in_names, out_names, out_avals, zero_shapes = [], [], [], []
        for alloc in nc.m.functions[0].allocations:
            if not isinstance(alloc, mybir.MemoryLocationSet):
                continue
            name = alloc.memorylocations[0].name
            if alloc.kind == "ExternalInput":
                if name != partition_name:
                    in_names.append(name)
            elif alloc.kind == "ExternalOutput":
                out_names.append(name)
                shape = tuple(alloc.tensor_shape)
                dtype = mybir.dt.np(alloc.dtype)
                out_avals.append(jax.core.ShapedArray(shape, dtype))
                zero_shapes.append((shape, dtype))
        self.in_names = in_names
        self.out_names = out_names
        n_params = len(in_names)
        n_outs = len(out_avals)
        all_in = list(in_names) + list(out_names)
        if partition_name is not None:
            all_in.append(partition_name)
        donate = tuple(range(n_params, n_params + n_outs))

        def _body(*args):
            operands = list(args)
            if partition_name is not None:
                operands.append(partition_id_tensor())
            outs = _bass_exec_p.bind(
                *operands,
                out_avals=tuple(out_avals),
                in_names=tuple(all_in),
                out_names=tuple(out_names),
                lowering_input_output_aliases=(),
                sim_require_finite=False,
                sim_require_nnan=False,
                nc=nc,
            )
            return tuple(outs)

        devices = jax.devices()[:n_cores]
        mesh = Mesh(np.asarray(devices), ("core",))
        in_specs = (PartitionSpec("core"),) * (n_params + n_outs)
        out_specs = (PartitionSpec("core"),) * n_outs
        self.sharded = jax.jit(
            shard_map(
                _body, mesh=mesh, in_specs=in_specs, out_specs=out_specs,
                check_rep=False,
            ),
            donate_argnums=donate,
            keep_unused=True,
        )
        shd = NamedSharding(mesh, PartitionSpec("core"))
        self.zeros_fn = jax.jit(
            lambda: tuple(
                jnp.zeros((n_cores * s[0], *s[1:]), d) for s, d in zero_shapes
            ),
            out_shardings=(shd,) * n_outs,
        )

        self._shd = shd
        self._devices = devices
        self.n_cores = n_cores
        from concurrent.futures import ThreadPoolExecutor

        self._pool = ThreadPoolExecutor(8)

    def put(self, arr):
        import jax

        return jax.device_put(arr, self._shd)

    def put_sharded_async(self, arrs):
        """arrs: list of np arrays shaped [NC*rows, ...]. Returns a callable
        that joins and yields the assembled global jax arrays."""
        import jax

        n = self.n_cores
        futs = []
        metas = []
        for arr in arrs:
            rows = arr.shape[0] // n
            shards = [arr[i * rows : (i + 1) * rows] for i in range(n)]
            fs = [
                self._pool.submit(jax.device_put, shards[i], self._devices[i])
                for i in range(n)
            ]
            futs.append(fs)
            metas.append(arr.shape)
        def join():
            out = []
            for fs, shape in zip(futs, metas):
                parts = [f.result() for f in fs]
                out.append(
                    jax.make_array_from_single_device_arrays(
                        shape, self._shd, parts
                    )
                )
            return out
        return join

    def fetch_np(self, arrs):
        """Fetch jax arrays via per-shard threaded copies; returns np arrays."""
        import numpy as _np

        def one_shard(s):
            return _np.asarray(s.data)

        all_futs = []
        for arr in arrs:
            shards = sorted(
                arr.addressable_shards, key=lambda s: s.index[0].start or 0
            )
            all_futs.append([self._pool.submit(one_shard, s) for s in shards])
        return [
            _np.concatenate([f.result() for f in fs], axis=0) for fs in all_futs
        ]

    def __call__(self, stacked_inputs):
        args = [stacked_inputs[n] for n in self.in_names]
        zeros = self.zeros_fn()
        outs = self.sharded(*args, *zeros)
        return dict(zip(self.out_names, outs))


# --------------------------------------------------------------------------
# fallback: pure-host computation (only if capacity asserts fail)
# --------------------------------------------------------------------------

def _host_fallback(feat, W_src, b_src, W_dst, b_dst, user_ids, item_ids,
                   edge_src, edge_dst):
    H_src = feat[user_ids]
    H_dst = feat[item_ids]
    alpha = np.einsum(
        "ed,ed->e", H_src[edge_src], H_dst[edge_dst], optimize=True
    ) * SCALE
    w = np.exp(alpha - alpha.max())
    w /= w.sum()
    FS = np.maximum(H_src @ W_src.T + b_src[None, :], 0.0)
    FD = np.maximum(H_dst @ W_dst.T + b_dst[None, :], 0.0)

    def seg_sum(vals, seg, nseg):
        out = np.zeros((nseg, D), np.float32)
        np.add.at(out, seg, vals)
        return out

    item_new = seg_sum(FS[edge_src] * w[:, None], edge_dst, len(item_ids))
    user_new = seg_sum(FD[edge_dst] * w[:, None], edge_src, len(user_ids))
    return np.concatenate([user_new, item_new], 0).astype(np.float32)


# --------------------------------------------------------------------------
# entry point
# --------------------------------------------------------------------------

def kernel(**inputs):
    import threading

    cfg = FULL
    feat = np.asarray(inputs["feat"], np.float32)
    W_src = np.asarray(inputs["W_src"], np.float32)
    b_src = np.asarray(inputs["b_src"], np.float32)
    W_dst = np.asarray(inputs["W_dst"], np.float32)
    b_dst = np.asarray(inputs["b_dst"], np.float32)
    user_ids = np.asarray(inputs["user_ids"]).astype(np.int64)
    item_ids = np.asarray(inputs["item_ids"]).astype(np.int64)
    edge_src = np.asarray(inputs["edge_src"]).astype(np.int64)
    edge_dst = np.asarray(inputs["edge_dst"]).astype(np.int64)

    f = host_prep_features(cfg, feat, user_ids, item_ids)
    if f is None:
        return _host_fallback(
            feat, W_src, b_src, W_dst, b_dst, user_ids, item_ids,
            edge_src, edge_dst,
        )
    hs_sh, hd_sh, inv_u, inv_i = f

    if "runner" not in _compiled:
        nc = build(cfg)
        _compiled["runner"] = SpmdRunner(nc, NC)
    runner = _compiled["runner"]

    # upload the big feature tables while the host builds edge indices
    join = runner.put_sharded_async([hs_sh, hd_sh])
    ins = host_prep_indices(
        cfg, inv_u, inv_i, W_src, b_src, W_dst, b_dst, edge_src, edge_dst
    )
    if ins is None:
        return _host_fallback(
            feat, W_src, b_src, W_dst, b_dst, user_ids, item_ids,
            edge_src, edge_dst,
        )
    ins["hs"], ins["hd"] = join()

    outs = runner(ins)
    LAST["results"] = None

    uo_f, io_f, us_f, is_f = runner.fetch_np(
        [outs["uo"], outs["io"], outs["us"], outs["is_"]]
    )
    uo = uo_f.reshape(NC, cfg.nt_u * P, D)[:, : cfg.upc]
    us = us_f.reshape(NC, cfg.nt_u * P, 1)[:, : cfg.upc]
    io = io_f.reshape(NC, cfg.nt_i * P, D)[:, : cfg.ipc]
    is_ = is_f.reshape(NC, cfg.nt_i * P, 1)[:, : cfg.ipc]
    user_new = uo.astype(np.float32) * us
    item_new = io.astype(np.float32) * is_
    return np.concatenate(
        [user_new.reshape(-1, D), item_new.reshape(-1, D)], 0
    )


# revision 11
# speedup vs baseline: 8.3717x; 1.0208x over previous
"""Trainium2 Bass kernel for the bipartite GNN message-passing layer.

All compute runs on the 8 NeuronCores:
  - node features are uploaded row-sharded in bf16 and AllGathered on-device
  - dense transforms (relu(H @ W^T + b)) run row-sharded on the PE array
  - edge dot-product attention (global softmax) and the alpha-weighted
    segment sums run edge-sharded: each core owns the edges whose
    destination (resp. source) it owns, so aggregation needs no collective
  - segment sums are computed as one-hot matmuls accumulated in PSUM over
    destination tiles (host pre-sorts edges by destination tile with fixed
    per-tile capacity), so no scatter-add races
  - outputs leave the device in bf16 to halve D2H traffic

Host does only O(E) integer index preprocessing and the initial
feat[user_ids]/feat[item_ids] row gathers.
"""
import os
import sys
from dataclasses import dataclass

import numpy as np

for _p in ("/opt/trn_rl_repo",):
    if _p not in sys.path and os.path.isdir(_p):
        sys.path.insert(0, _p)

import ml_dtypes

BF16 = ml_dtypes.bfloat16
P = 128
D = 256
NC = 8
SCALE = 1.0 / 16.0


@dataclass(frozen=True)
class Cfg:
    n_nodes: int
    upc: int      # users per core
    upadc: int    # padded users per core (multiple of 128)
    ipc: int      # items per core
    ipadc: int    # padded items per core (multiple of 128)
    nt_i: int     # item output tiles per core
    cpt_i: int    # chunks (128 edges) per item tile
    nt_u: int     # user output tiles per core (>= ceil(upc/128))
    cpt_u: int
    blk: int      # edges per gather block (multiple of 128)
    gtu: int      # deduped user-node gather-table rows per core (mult of 128)
    gti: int      # deduped item-node gather-table rows per core (mult of 128)

    @property
    def ec_i(self):
        return self.nt_i * self.cpt_i * P

    @property
    def ec_u(self):
        return self.nt_u * self.cpt_u * P

    @property
    def nblk_i(self):
        return self.ec_i // self.blk

    @property
    def nblk_u(self):
        return self.ec_u // self.blk

    @property
    def sb(self):
        return self.blk // P


FULL = Cfg(
    n_nodes=70000,
    upc=6250, upadc=6272, ipc=2500, ipadc=2560,
    nt_i=20, cpt_i=20, nt_u=50, cpt_u=8, blk=2048,
    gtu=4608, gti=2304,
)

_compiled = {}
LAST = {}


# --------------------------------------------------------------------------
# device kernel
# --------------------------------------------------------------------------

def build(cfg: Cfg):
    import concourse.bacc as bacc
    import concourse.mybir as mybir
    import concourse.tile as tile
    import concourse.bass_isa as bass_isa

    f32 = mybir.dt.float32
    bf16 = mybir.dt.bfloat16
    i16 = mybir.dt.int16
    i32 = mybir.dt.int32
    u8 = mybir.dt.uint8
    Alu = mybir.AluOpType
    Act = mybir.ActivationFunctionType
    SB = cfg.sb

    nc = bacc.Bacc("TRN2", target_bir_lowering=False, debug=False, num_devices=NC)

    t_hs = nc.dram_tensor("hs", [cfg.gtu, D], bf16, kind="ExternalInput")
    t_hd = nc.dram_tensor("hd", [cfg.gti, D], bf16, kind="ExternalInput")
    t_wst = nc.dram_tensor("wst", [2 * P, D], bf16, kind="ExternalInput")
    t_wdt = nc.dram_tensor("wdt", [2 * P, D], bf16, kind="ExternalInput")
    t_bs = nc.dram_tensor("bs", [1, D], f32, kind="ExternalInput")
    t_bd = nc.dram_tensor("bd", [1, D], f32, kind="ExternalInput")
    # item-direction (edges sharded by destination-item owner, sorted by tile)
    t_ihs = nc.dram_tensor("ihs", [16, cfg.ec_i // 16], i16, kind="ExternalInput")
    t_ihd = nc.dram_tensor("ihd", [16, cfg.ec_i // 16], i16, kind="ExternalInput")
    t_ipar = nc.dram_tensor("ipar", [P, cfg.ec_i // P], bf16, kind="ExternalInput")
    t_ival = nc.dram_tensor("ival", [P, cfg.ec_i // P], bf16, kind="ExternalInput")
    t_idm = nc.dram_tensor("idm", [P, cfg.ec_i // P], u8, kind="ExternalInput")
    # user-direction
    t_uhs = nc.dram_tensor("uhs", [16, cfg.ec_u // 16], i16, kind="ExternalInput")
    t_uhd = nc.dram_tensor("uhd", [16, cfg.ec_u // 16], i16, kind="ExternalInput")
    t_upar = nc.dram_tensor("upar", [P, cfg.ec_u // P], bf16, kind="ExternalInput")
    t_uval = nc.dram_tensor("uval", [P, cfg.ec_u // P], bf16, kind="ExternalInput")
    t_udm = nc.dram_tensor("udm", [P, cfg.ec_u // P], u8, kind="ExternalInput")

    i8 = mybir.dt.int8
    t_uo = nc.dram_tensor("uo", [cfg.nt_u * P, D], i8, kind="ExternalOutput")
    t_io = nc.dram_tensor("io", [cfg.nt_i * P, D], i8, kind="ExternalOutput")
    t_us = nc.dram_tensor("us", [cfg.nt_u * P, 1], f32, kind="ExternalOutput")
    t_is = nc.dram_tensor("is_", [cfg.nt_i * P, 1], f32, kind="ExternalOutput")

    CH_I = cfg.nt_i * cfg.cpt_i
    CH_U = cfg.nt_u * cfg.cpt_u

    with tile.TileContext(nc) as tc:
        with (
            tc.tile_pool(name="const", bufs=1) as cp,
            tc.tile_pool(name="idx", bufs=1) as ip,
            tc.tile_pool(name="dram", bufs=1, space="DRAM") as dr,
            tc.tile_pool(name="ps", bufs=2, space="PSUM") as pp,
            tc.tile_pool(name="gth", bufs=1) as gp_,
            tc.tile_pool(name="wrk", bufs=1) as wp,
            tc.tile_pool(name="out", bufs=2) as op_,
        ):
            # ---------------- constants / index staging ----------------
            iota_i = cp.tile([P, P], i32, tag="iota_i")
            nc.gpsimd.iota(iota_i[:], pattern=[[1, P]], base=0, channel_multiplier=0)
            iota_f = cp.tile([P, P], f32, tag="iota_f")
            nc.vector.tensor_copy(iota_f[:], iota_i[:])

            wst0 = cp.tile([P, D], bf16, tag="wst0")
            wst1 = cp.tile([P, D], bf16, tag="wst1")
            wdt0 = cp.tile([P, D], bf16, tag="wdt0")
            wdt1 = cp.tile([P, D], bf16, tag="wdt1")
            nc.sync.dma_start(out=wst0[:], in_=t_wst[0:P, :])
            nc.sync.dma_start(out=wst1[:], in_=t_wst[P : 2 * P, :])
            nc.sync.dma_start(out=wdt0[:], in_=t_wdt[0:P, :])
            nc.sync.dma_start(out=wdt1[:], in_=t_wdt[P : 2 * P, :])

            bias = {}
            for key, tb in (("s", t_bs), ("d", t_bd)):
                b1 = cp.tile([1, D], f32, tag=f"b1{key}")
                nc.sync.dma_start(out=b1[:], in_=tb[:])
                bb = cp.tile([P, D], f32, tag=f"bb{key}")
                nc.gpsimd.partition_broadcast(bb[:], b1[:])
                bias[key] = bb

            def stage_idx(th, n, tag):
                t = ip.tile([P, n // 16], i16, tag=tag)
                for k in range(8):
                    nc.sync.dma_start(out=t[16 * k : 16 * (k + 1), :], in_=th[:, :])
                return t

            ihs = stage_idx(t_ihs, cfg.ec_i, "ihs")
            ihd = stage_idx(t_ihd, cfg.ec_i, "ihd")
            uhs = stage_idx(t_uhs, cfg.ec_u, "uhs")
            uhd = stage_idx(t_uhd, cfg.ec_u, "uhd")

            def stage_pl(th, n, dt, tag):
                t = ip.tile([P, n // P], dt, tag=tag)
                nc.sync.dma_start(out=t[:], in_=th[:, :])
                return t

            ipar = stage_pl(t_ipar, cfg.ec_i, bf16, "ipar")
            ival = stage_pl(t_ival, cfg.ec_i, bf16, "ival")
            idm8 = stage_pl(t_idm, cfg.ec_i, u8, "idm8")
            upar = stage_pl(t_upar, cfg.ec_u, bf16, "upar")
            uval = stage_pl(t_uval, cfg.ec_u, bf16, "uval")
            udm8 = stage_pl(t_udm, cfg.ec_u, u8, "udm8")

            idmf = ip.tile([P, CH_I], f32, tag="idmf")
            nc.vector.tensor_copy(idmf[:], idm8[:])
            udmf = ip.tile([P, CH_U], f32, tag="udmf")
            nc.vector.tensor_copy(udmf[:], udm8[:])

            # ---------------- AllGather H tables ----------------
            hs_b = dr.tile([cfg.gtu, D], bf16, tag="hs_b")
            hd_b = dr.tile([cfg.gti, D], bf16, tag="hd_b")
            nc.gpsimd.dma_start(out=hs_b[:], in_=t_hs[:])
            nc.gpsimd.dma_start(out=hd_b[:], in_=t_hd[:])
            HsF = dr.tile([NC * cfg.gtu, D], bf16, tag="HsF")
            HdF = dr.tile([NC * cfg.gti, D], bf16, tag="HdF")
            nc.gpsimd.collective_compute(
                "AllGather", Alu.bypass, replica_groups=[list(range(NC))],
                ins=[hs_b[:].opt()], outs=[HsF[:].opt()],
            )
            nc.gpsimd.collective_compute(
                "AllGather", Alu.bypass, replica_groups=[list(range(NC))],
                ins=[hd_b[:].opt()], outs=[HdF[:].opt()],
            )
            HsP = HsF[:].rearrange("(a b) c -> a (b c)", b=2)
            HdA = HdF[:]

            # ---------------- dense transforms ----------------
            fs_sh = dr.tile([cfg.gtu, D], bf16, tag="fs_sh")
            fd_sh = dr.tile([cfg.gti, D], bf16, tag="fd_sh")

            for key, t_in, npad, w0, w1, f_out in (
                ("s", t_hs, cfg.gtu, wst0, wst1, fs_sh),
                ("d", t_hd, cfg.gti, wdt0, wdt1, fd_sh),
            ):
                htA = cp.tile([P, npad], bf16, tag=f"htA{key}")
                htB = cp.tile([P, npad], bf16, tag=f"htB{key}")
                nc.sync.dma_start_transpose(htA[:], t_in[:, 0:P])
                nc.sync.dma_start_transpose(htB[:], t_in[:, P : 2 * P])
                for t in range(npad // P):
                    ps = pp.tile([P, D], f32, tag="mmps")
                    nc.tensor.matmul(
                        out=ps[:], lhsT=htA[:, t * P : (t + 1) * P], rhs=w0[:],
                        start=True, stop=False,
                    )
                    nc.tensor.matmul(
                        out=ps[:], lhsT=htB[:, t * P : (t + 1) * P], rhs=w1[:],
                        start=False, stop=True,
                    )
                    tmp = op_.tile([P, D], f32, tag="mmtmp")
                    nc.vector.tensor_tensor(tmp[:], ps[:], bias[key][:], Alu.add)
                    ft = op_.tile([P, D], bf16, tag="mmft")
                    nc.vector.tensor_scalar_max(ft[:], tmp[:], 0.0)
                    nc.sync.dma_start(out=f_out[t * P : (t + 1) * P, :], in_=ft[:])

            FsF = dr.tile([NC * cfg.gtu, D], bf16, tag="FsF")
            FdF = dr.tile([NC * cfg.gti, D], bf16, tag="FdF")
            nc.gpsimd.collective_compute(
                "AllGather", Alu.bypass, replica_groups=[list(range(NC))],
                ins=[fs_sh[:].opt()], outs=[FsF[:].opt()],
            )
            nc.gpsimd.collective_compute(
                "AllGather", Alu.bypass, replica_groups=[list(range(NC))],
                ins=[fd_sh[:].opt()], outs=[FdF[:].opt()],
            )
            FsP = FsF[:].rearrange("(a b) c -> a (b c)", b=2)
            FdA = FdF[:]

            # ---------------- item-direction alphas ----------------
            alpha_i = ip.tile([P, CH_I], f32, tag="alpha_i")

            GCALL = 1024  # max idxs per dma_gather (16-DMA ring: 128 descs)

            def emit_gather(dst, table, idxt, base_col, blk, elem):
                n = 0
                while n < blk:
                    step = min(GCALL, blk - n)
                    nc.gpsimd.dma_gather(
                        dst[:, n // P : (n + step) // P, :],
                        table,
                        idxt[:, base_col + n // 16 : base_col + (n + step) // 16],
                        step, step, elem,
                    )
                    n += step

            def pair_select(gpt, par_sl, tag):
                """gpt [P,SB,2D] pair-gather; returns selected [P,SB,D] bf16."""
                sel = wp.tile([P, SB, D], bf16, tag=tag)
                nc.vector.tensor_tensor(
                    sel[:], gpt[:, :, D : 2 * D], gpt[:, :, 0:D], Alu.subtract
                )
                nc.vector.tensor_tensor(
                    sel[:], sel[:],
                    par_sl[:, :, None].to_broadcast((P, SB, D)), Alu.mult,
                )
                nc.vector.tensor_tensor(sel[:], sel[:], gpt[:, :, 0:D], Alu.add)
                return sel

            for b in range(cfg.nblk_i):
                i16sl = slice(b * (cfg.blk // 16), (b + 1) * (cfg.blk // 16))
                chsl = slice(b * SB, (b + 1) * SB)
                gpt = gp_.tile([P, SB, 2 * D], bf16, tag="gp")
                emit_gather(gpt, HsP, ihs, b * (cfg.blk // 16), cfg.blk, 2 * D)
                sel = pair_select(gpt, ipar[:, chsl], "sel")
                gb = gp_.tile([P, SB, D], bf16, tag="gb")
                emit_gather(gb, HdA, ihd, b * (cfg.blk // 16), cfg.blk, D)
                nc.vector.tensor_tensor(gb[:], sel[:], gb[:], Alu.mult)
                nc.vector.tensor_reduce(
                    alpha_i[:, chsl], gb[:], mybir.AxisListType.X, Alu.add
                )

            # ---------------- global softmax stats ----------------
            lred = cp.tile([P, 1], f32, tag="lred")
            pred = cp.tile([P, 1], f32, tag="pred")
            nc.vector.tensor_reduce(
                lred[:], alpha_i[:], mybir.AxisListType.X, Alu.max
            )
            nc.gpsimd.partition_all_reduce(
                pred[:], lred[:], channels=P, reduce_op=bass_isa.ReduceOp.max
            )
            cc_in = dr.tile([1, 1], f32, tag="cc_in")
            cc_out = dr.tile([1, 1], f32, tag="cc_out")
            nc.gpsimd.dma_start(out=cc_in[:], in_=pred[0:1, 0:1])
            nc.gpsimd.collective_compute(
                "AllReduce", Alu.max, replica_groups=[list(range(NC))],
                ins=[cc_in[:].opt()], outs=[cc_out[:].opt()],
            )
            gmax1 = cp.tile([1, 1], f32, tag="gmax1")
            nc.sync.dma_start(out=gmax1[:], in_=cc_out[:])
            negb = cp.tile([P, 1], f32, tag="negb")
            nc.gpsimd.partition_broadcast(negb[:], gmax1[:])
            nc.vector.tensor_scalar_mul(negb[:], negb[:], -SCALE)

            w_i = ip.tile([P, CH_I], f32, tag="w_i")
            nc.scalar.activation(
                w_i[:], alpha_i[:], Act.Exp, bias=negb[:], scale=SCALE
            )
            nc.vector.tensor_tensor(w_i[:], w_i[:], ival[:], Alu.mult)
            lsum = cp.tile([P, 1], f32, tag="lsum")
            psum_ = cp.tile([P, 1], f32, tag="psum_")
            nc.vector.tensor_reduce(
                lsum[:], w_i[:], mybir.AxisListType.X, Alu.add
            )
            nc.gpsimd.partition_all_reduce(
                psum_[:], lsum[:], channels=P, reduce_op=bass_isa.ReduceOp.add
            )
            z_in = dr.tile([1, 1], f32, tag="z_in")
            z_out = dr.tile([1, 1], f32, tag="z_out")
            nc.gpsimd.dma_start(out=z_in[:], in_=psum_[0:1, 0:1])
            nc.gpsimd.collective_compute(
                "AllReduce", Alu.add, replica_groups=[list(range(NC))],
                ins=[z_in[:].opt()], outs=[z_out[:].opt()],
            )
            zt = cp.tile([1, 1], f32, tag="zt")
            nc.sync.dma_start(out=zt[:], in_=z_out[:])
            invz1 = cp.tile([1, 1], f32, tag="invz1")
            nc.vector.reciprocal(invz1[:], zt[:])
            invz = cp.tile([P, 1], f32, tag="invz")
            nc.gpsimd.partition_broadcast(invz[:], invz1[:])
            nc.vector.tensor_tensor(
                w_i[:], w_i[:], invz[:].to_broadcast((P, CH_I)), Alu.mult
            )

            # ---------------- item-direction aggregation ----------------
            MAGIC = 12582912.0  # 1.5 * 2**23: add/sub forces RNE to integer

            def agg_blocks(nblk, cpt, w_src, dm_f, sel_fn, t_out, t_scale, tag_pfx):
                ps = None
                for b in range(nblk):
                    sel, wsl = sel_fn(b)
                    eqw = wp.tile([P, SB, P], bf16, tag=f"{tag_pfx}eqw")
                    nc.vector.tensor_tensor(
                        eqw[:],
                        dm_f[:, b * SB : (b + 1) * SB, None].to_broadcast((P, SB, P)),
                        iota_f[:, None, :].to_broadcast((P, SB, P)),
                        Alu.is_equal,
                    )
                    nc.vector.tensor_tensor(
                        eqw[:], eqw[:], wsl, Alu.mult
                    )
                    for sl in range(SB):
                        g = b * SB + sl
                        t, c = divmod(g, cpt)
                        if c == 0:
                            ps = pp.tile([P, D], f32, tag=f"{tag_pfx}ps")
                        nc.tensor.matmul(
                            out=ps[:], lhsT=eqw[:, sl, :], rhs=sel[:, sl, :],
                            start=(c == 0), stop=(c == cpt - 1),
                        )
                        if c == cpt - 1:
                            am = op_.tile([P, 1], f32, tag=f"{tag_pfx}am")
                            nc.vector.tensor_reduce(
                                am[:], ps[:], mybir.AxisListType.X, Alu.max,
                                apply_absolute_value=True,
                            )
                            nc.vector.tensor_scalar_add(am[:], am[:], 1e-30)
                            dsc = op_.tile([P, 1], f32, tag=f"{tag_pfx}dsc")
                            nc.vector.tensor_scalar_mul(dsc[:], am[:], 1.0 / 127.0)
                            nc.sync.dma_start(
                                out=t_scale[t * P : (t + 1) * P, :], in_=dsc[:]
                            )
                            k = op_.tile([P, 1], f32, tag=f"{tag_pfx}k")
                            nc.vector.reciprocal(k[:], am[:])
                            nc.vector.tensor_scalar_mul(k[:], k[:], 127.0)
                            sq = op_.tile([P, D], f32, tag=f"{tag_pfx}sq")
                            nc.scalar.activation(
                                sq[:], ps[:], Act.Copy, bias=MAGIC, scale=k[:]
                            )
                            nc.vector.tensor_scalar_sub(sq[:], sq[:], MAGIC)
                            ob = op_.tile([P, D], i8, tag=f"{tag_pfx}ob")
                            nc.vector.tensor_copy(ob[:], sq[:])
                            nc.sync.dma_start(
                                out=t_out[t * P : (t + 1) * P, :], in_=ob[:]
                            )

            def item_sel(b):
                i16sl = slice(b * (cfg.blk // 16), (b + 1) * (cfg.blk // 16))
                chsl = slice(b * SB, (b + 1) * SB)
                gpt = gp_.tile([P, SB, 2 * D], bf16, tag="gp")
                emit_gather(gpt, FsP, ihs, b * (cfg.blk // 16), cfg.blk, 2 * D)
                sel = pair_select(gpt, ipar[:, chsl], "sel")
                wsl = w_i[:, chsl, None].to_broadcast((P, SB, P))
                return sel, wsl

            agg_blocks(cfg.nblk_i, cfg.cpt_i, w_i, idmf, item_sel, t_io, t_is, "i")

            # ---------------- user-direction (alpha fused) ----------------
            w_ub = {}

            def user_sel(b):
                i16sl = slice(b * (cfg.blk // 16), (b + 1) * (cfg.blk // 16))
                chsl = slice(b * SB, (b + 1) * SB)
                gpt = gp_.tile([P, SB, 2 * D], bf16, tag="gp")
                emit_gather(gpt, HsP, uhs, b * (cfg.blk // 16), cfg.blk, 2 * D)
                sel = pair_select(gpt, upar[:, chsl], "sel")
                gb = gp_.tile([P, SB, D], bf16, tag="gb")
                emit_gather(gb, HdA, uhd, b * (cfg.blk // 16), cfg.blk, D)
                nc.vector.tensor_tensor(gb[:], sel[:], gb[:], Alu.mult)
                aub = wp.tile([P, SB], f32, tag="aub")
                nc.vector.tensor_reduce(
                    aub[:], gb[:], mybir.AxisListType.X, Alu.add
                )
                wub = wp.tile([P, SB], f32, tag="wub")
                nc.scalar.activation(
                    wub[:], aub[:], Act.Exp, bias=negb[:], scale=SCALE
                )
                nc.vector.tensor_tensor(
                    wub[:], wub[:], uval[:, chsl], Alu.mult
                )
                nc.vector.tensor_tensor(
                    wub[:], wub[:], invz[:].to_broadcast((P, SB)), Alu.mult
                )
                gfd = gp_.tile([P, SB, D], bf16, tag="gfd")
                emit_gather(gfd, FdA, uhd, b * (cfg.blk // 16), cfg.blk, D)
                wsl = wub[:, :, None].to_broadcast((P, SB, P))
                return gfd, wsl

            agg_blocks(cfg.nblk_u, cfg.cpt_u, None, udmf, user_sel, t_uo, t_us, "u")

    nc.finalize()
    return nc


# --------------------------------------------------------------------------
# host preprocessing
# --------------------------------------------------------------------------

def wrap16(a):
    """per-edge int array [NCORES, n] -> dma_gather 16-wrap layout [NC,16,n/16]."""
    ncore, n = a.shape
    return np.ascontiguousarray(
        a.reshape(ncore, n // 16, 16).transpose(0, 2, 1)
    ).astype(np.int16)


def glayout(a):
    """per-edge array [NCORES, n] -> gather-output layout [NC, 128, n/128]."""
    ncore, n = a.shape
    return np.ascontiguousarray(a.reshape(ncore, n // P, P).transpose(0, 2, 1))


def prep_direction(cfg: Cfg, own_ids, own_per_core, nt, cpt, hs_row, hd_row):
    """Sort/pad one direction's edges by (owner core, dest tile).

    own_ids: destination-side node ids (ownership + one-hot row)
    hs_row/hd_row: per-edge rows into the deduped gather tables
    """
    E = own_ids.shape[0]
    ec = nt * cpt * P
    own = own_ids // own_per_core
    loc = own_ids - own * own_per_core
    til = loc >> 7
    grp = own * nt + til
    order = np.argsort(grp, kind="stable")
    grp_s = grp[order]
    counts = np.bincount(grp, minlength=NC * nt)
    if counts.max() > cpt * P:
        return None
    starts = np.concatenate([[0], np.cumsum(counts)[:-1]])
    rank = np.arange(E, dtype=np.int64) - starts[grp_s]
    slot = (grp_s % nt) * (cpt * P) + rank
    core = grp_s // nt
    loc_s = loc[order]
    hs_s = hs_row[order]
    hd_s = hd_row[order]

    hs_idx = np.zeros((NC, ec), np.int32)
    hd_idx = np.zeros((NC, ec), np.int32)
    par = np.zeros((NC, ec), np.float32)
    val = np.zeros((NC, ec), np.float32)
    dmod = np.zeros((NC, ec), np.int32)
    hs_idx[core, slot] = hs_s >> 1
    par[core, slot] = hs_s & 1
    hd_idx[core, slot] = hd_s
    dmod[core, slot] = loc_s & 127
    val[core, slot] = 1.0
    return {
        "hs": wrap16(hs_idx),
        "hd": wrap16(hd_idx),
        "par": glayout(par).astype(BF16),
        "val": glayout(val).astype(BF16),
        "dm": glayout(dmod).astype(np.uint8),
    }


def host_prep_features(cfg: Cfg, feat, user_ids, item_ids):
    uu, inv_u = np.unique(user_ids, return_inverse=True)
    ii, inv_i = np.unique(item_ids, return_inverse=True)
    if len(uu) > NC * cfg.gtu or len(ii) > NC * cfg.gti:
        return None
    hs_sh = np.zeros((NC * cfg.gtu, D), BF16)
    hs_sh[: len(uu)] = feat[uu].astype(BF16)
    hd_sh = np.zeros((NC * cfg.gti, D), BF16)
    hd_sh[: len(ii)] = feat[ii].astype(BF16)
    return hs_sh, hd_sh, inv_u, inv_i


def host_prep_indices(cfg: Cfg, inv_u, inv_i, W_src, b_src, W_dst, b_dst,
                      edge_src, edge_dst):
    e_hs = inv_u[edge_src].astype(np.int64)
    e_hd = inv_i[edge_dst].astype(np.int64)

    idir = prep_direction(
        cfg, edge_dst, cfg.ipc, cfg.nt_i, cfg.cpt_i, e_hs, e_hd
    )
    udir = prep_direction(
        cfg, edge_src, cfg.upc, cfg.nt_u, cfg.cpt_u, e_hs, e_hd
    )
    if idir is None or udir is None:
        return None

    wst = np.ascontiguousarray(W_src.T).astype(BF16)
    wdt = np.ascontiguousarray(W_dst.T).astype(BF16)

    def rep(a):
        return np.broadcast_to(a, (NC, *a.shape))

    ins = {
        "wst": rep(wst), "wdt": rep(wdt),
        "bs": rep(b_src.reshape(1, D).astype(np.float32)),
        "bd": rep(b_dst.reshape(1, D).astype(np.float32)),
        "ihs": idir["hs"], "ihd": idir["hd"], "ipar": idir["par"],
        "ival": idir["val"], "idm": idir["dm"],
        "uhs": udir["hs"], "uhd": udir["hd"], "upar": udir["par"],
        "uval": udir["val"], "udm": udir["dm"],
    }
    return {k: np.ascontiguousarray(v.reshape(-1, *v.shape[2:])) for k, v in ins.items()}


def host_prep(cfg: Cfg, feat, user_ids, item_ids, W_src, b_src, W_dst, b_dst,
              edge_src, edge_dst):
    """Non-overlapped variant (used by the sim tests)."""
    f = host_prep_features(cfg, feat, user_ids, item_ids)
    if f is None:
        return None
    hs_sh, hd_sh, inv_u, inv_i = f
    ins = host_prep_indices(
        cfg, inv_u, inv_i, W_src, b_src, W_dst, b_dst, edge_src, edge_dst
    )
    if ins is None:
        return None
    ins["hs"] = hs_sh
    ins["hd"] = hd_sh
    return ins


# --------------------------------------------------------------------------
# cached SPMD runner (jit built once, zeros created on-device)
# --------------------------------------------------------------------------

class SpmdRunner:
    def __init__(self, nc, n_cores):
        import jax
        import jax.numpy as jnp
        from jax.sharding import Mesh, NamedSharding, PartitionSpec
        from jax.experimental.shard_map import shard_map
        from concourse import mybir
        from concourse.bass2jax import (
            _bass_exec_p, partition_id_tensor, install_neuronx_cc_hook,
        )

        install_neuronx_cc_hook()
        partition_name = (
            nc.partition_id_tensor.name if nc.partition_id_tensor else None
        )
        in_names, out_names, out_avals, zero_shapes = [], [], [], []
        for alloc in nc.m.functions[0].allocations:
            if not isinstance(alloc, mybir.MemoryLocationSet):
                continue
            name = alloc.memorylocations[0].name
            if alloc.kind == "ExternalInput":
                if name != partition_name:
                    in_names.append(name)
            elif alloc.kind == "ExternalOutput":
                out_names.append(name)
                shape = tuple(alloc.tensor_shape)
                dtype = mybir.dt.np(alloc.dtype)
                out_avals.append(jax.core.ShapedArray(shape, dtype))
                zero_shapes.append((shape, dtype))
        self.in_names = in_names
        self.out_names = out_names
        n_params = len(in_names)
        n_outs = len(out_avals)
        all_in = list(in_names) + list(out_names)
        if partition_name is not None:
            all_in.append(partition_name)
        donate = tuple(range(n_params, n_params + n_outs))

        def _body(*args):
            operands = list(args)
            if partition_name is not None:
                operands.append(partition_id_tensor())
            outs = _bass_exec_p.bind(
                *operands,
                out_avals=tuple(out_avals),
                in_names=tuple(all_in),
                out_names=tuple(out_names),
                lowering_input_output_aliases=(),
                sim_require_finite=False,
                sim_require_nnan=False,
                nc=nc,
            )
            return tuple(outs)

        devices = jax.devices()[:n_cores]
        mesh = Mesh(np.asarray(devices), ("core",))
        in_specs = (PartitionSpec("core"),) * (n_params + n_outs)
        out_specs = (PartitionSpec("core"),) * n_outs
        self.sharded = jax.jit(
            shard_map(
                _body, mesh=mesh, in_specs=in_specs, out_specs=out_specs,
                check_rep=False,
            ),
            donate_argnums=donate,
            keep_unused=True,
        )
        shd = NamedSharding(mesh, PartitionSpec("core"))
        self.zeros_fn = jax.jit(
            lambda: tuple(
                jnp.zeros((n_cores * s[0], *s[1:]), d) for s, d in zero_shapes
            ),
            out_shardings=(shd,) * n_outs,
        )

        self._shd = shd
        self._devices = devices
        self.n_cores = n_cores
        from concurrent.futures import ThreadPoolExecutor

        self._pool = ThreadPoolExecutor(8)

    def put(self, arr):
        import jax

        return jax.device_put(arr, self._shd)

    def put_sharded_async(self, arrs):
        """arrs: list of np arrays shaped [NC*rows, ...]. Returns a callable
        that joins and yields the assembled global jax arrays."""
        import jax

        n = self.n_cores
        futs = []
        metas = []
        for arr in arrs:
            rows = arr.shape[0] // n
            shards = [arr[i * rows : (i + 1) * rows] for i in range(n)]
            fs = [
                self._pool.submit(jax.device_put, shards[i], self._devices[i])
                for i in range(n)
            ]
            futs.append(fs)
            metas.append(arr.shape)
        def join():
            out = []
            for fs, shape in zip(futs, metas):
                parts = [f.result() for f in fs]
                out.append(
                    jax.make_array_from_single_device_arrays(
                        shape, self._shd, parts
                    )
                )
            return out
        return join

    def fetch_np(self, arrs):
        """Fetch jax arrays via per-shard threaded copies; returns np arrays."""
        import numpy as _np

        def one_shard(s):
            return _np.asarray(s.data)

        all_futs = []
        for arr in arrs:
            shards = sorted(
                arr.addressable_shards, key=lambda s: s.index[0].start or 0
            )
            all_futs.append([self._pool.submit(one_shard, s) for s in shards])
        return [
            _np.concatenate([f.result() for f in fs], axis=0) for fs in all_futs
        ]

    def __call__(self, stacked_inputs):
        args = [stacked_inputs[n] for n in self.in_names]
        zeros = self.zeros_fn()
        outs = self.sharded(*args, *zeros)
        return dict(zip(self.out_names, outs))


# --------------------------------------------------------------------------
# fallback: pure-host computation (only if capacity asserts fail)
# --------------------------------------------------------------------------

def _host_fallback(feat, W_src, b_src, W_dst, b_dst, user_ids, item_ids,
                   edge_src, edge_dst):
    H_src = feat[user_ids]
    H_dst = feat[item_ids]
    alpha = np.einsum(
        "ed,ed->e", H_src[edge_src], H_dst[edge_dst], optimize=True
    ) * SCALE
    w = np.exp(alpha - alpha.max())
    w /= w.sum()
    FS = np.maximum(H_src @ W_src.T + b_src[None, :], 0.0)
    FD = np.maximum(H_dst @ W_dst.T + b_dst[None, :], 0.0)

    def seg_sum(vals, seg, nseg):
        out = np.zeros((nseg, D), np.float32)
        np.add.at(out, seg, vals)
        return out

    item_new = seg_sum(FS[edge_src] * w[:, None], edge_dst, len(item_ids))
    user_new = seg_sum(FD[edge_dst] * w[:, None], edge_src, len(user_ids))
    return np.concatenate([user_new, item_new], 0).astype(np.float32)


# --------------------------------------------------------------------------
# entry point
# --------------------------------------------------------------------------

def kernel(**inputs):
    import threading

    cfg = FULL
    feat = np.asarray(inputs["feat"], np.float32)
    W_src = np.asarray(inputs["W_src"], np.float32)
    b_src = np.asarray(inputs["b_src"], np.float32)
    W_dst = np.asarray(inputs["W_dst"], np.float32)
    b_dst = np.asarray(inputs["b_dst"], np.float32)
    user_ids = np.asarray(inputs["user_ids"]).astype(np.int64)
    item_ids = np.asarray(inputs["item_ids"]).astype(np.int64)
    edge_src = np.asarray(inputs["edge_src"]).astype(np.int64)
    edge_dst = np.asarray(inputs["edge_dst"]).astype(np.int64)

    f = host_prep_features(cfg, feat, user_ids, item_ids)
    if f is None:
        return _host_fallback(
            feat, W_src, b_src, W_dst, b_dst, user_ids, item_ids,
            edge_src, edge_dst,
        )
    hs_sh, hd_sh, inv_u, inv_i = f

    if "runner" not in _compiled:
        nc = build(cfg)
        _compiled["runner"] = SpmdRunner(nc, NC)
    runner = _compiled["runner"]

    # upload the big feature tables while the host builds edge indices
    staged = {}

    def _stage():
        staged["hs"] = runner.put(hs_sh)
        staged["hd"] = runner.put(hd_sh)

    th = threading.Thread(target=_stage)
    th.start()
    ins = host_prep_indices(
        cfg, inv_u, inv_i, W_src, b_src, W_dst, b_dst, edge_src, edge_dst
    )
    th.join()
    if ins is None:
        return _host_fallback(
            feat, W_src, b_src, W_dst, b_dst, user_ids, item_ids,
            edge_src, edge_dst,
        )
    ins["hs"] = staged["hs"]
    ins["hd"] = staged["hd"]

    outs = runner(ins)
    LAST["results"] = None

    fetched = {}

    def _fetch_small():
        fetched["io"] = np.asarray(outs["io"])
        fetched["us"] = np.asarray(outs["us"])
        fetched["is_"] = np.asarray(outs["is_"])

    th2 = threading.Thread(target=_fetch_small)
    th2.start()
    uo = np.asarray(outs["uo"]).reshape(NC, cfg.nt_u * P, D)[:, : cfg.upc]
    th2.join()
    us = fetched["us"].reshape(NC, cfg.nt_u * P, 1)[:, : cfg.upc]
    io = fetched["io"].reshape(NC, cfg.nt_i * P, D)[:, : cfg.ipc]
    is_ = fetched["is_"].reshape(NC, cfg.nt_i * P, 1)[:, : cfg.ipc]
    user_new = uo.astype(np.float32) * us
    item_new = io.astype(np.float32) * is_
    return np.concatenate(
        [user_new.reshape(-1, D), item_new.reshape(-1, D)], 0
    )
